# revision 25
# baseline (speedup 1.0000x reference)
"""Trainium2 Bass kernel for nn_MultiHeadConvNNAttention.

Sharding: 8 cores; core d handles batch b = d//2 and head-group g = d%2
(4 heads of H=8 each). Per core: q/k/v projections (fp32 on PE), per-head
SxS similarity (k^T @ qn, fp32), exact top-9 per row via segmented DVE
max8/max_index, neighbor-conv as 9 accumulating indirect-DMA gathers from a
precomputed u-table in DRAM, the torch-view output scramble via PE
transposes, and a partial Wo projection. Host sums the two partial y's per
batch and adds Wo_b.

Self-contained: hardcodes all shapes; does not read /root/problem files.
"""
import sys
import numpy as np

sys.path.insert(0, "/opt/trn_rl_repo")

import bass_rust
import concourse.bass as bass
import concourse.bacc as bacc_mod
import concourse.mybir as mybir
import concourse.tile as tile
from concourse.masks import make_identity
from contextlib import ExitStack

B, S, D, H, K = 4, 2048, 512, 8, 9
DK = D // H          # 64
HPD = H // 2         # heads per device = 4
NCHUNK = S // 128    # 16 i-chunks
NSEG = 8             # row segments for seg-max8
SEG = S // NSEG      # 256
FP32 = mybir.dt.float32
U16 = mybir.dt.uint16
I16 = mybir.dt.int16
U32 = mybir.dt.uint32
I32 = mybir.dt.int32
NEG = -3.0e38


def _split_multi_waits(nc):
    """This walrus build supports at most one sem-wait per instruction;
    split extras onto preceding same-engine drain carriers."""
    n = [0]

    def fix_block(blk):
        insts = blk.instructions
        out = []
        changed = False
        for i in insts:
            si = i.sync_info
            ow = list(si.on_wait) if si is not None and si.on_wait is not None else []
            if len(ow) > 1:
                changed = True
                for w in ow[:-1]:
                    n[0] += 1
                    c = mybir.InstDrain(name=f"wsplit_{n[0]}", ins=[], outs=[])
                    c.engine = i.engine
                    c.sync_info = bass_rust.SyncInfo(on_wait=[w], on_update=[])
                    out.append(c)
                i.sync_info = bass_rust.SyncInfo(
                    on_wait=[ow[-1]], on_update=list(si.on_update or []))
            out.append(i)
        if changed:
            blk.instructions = out

    for f in nc.m.functions:
        for blk in f.blocks:
            fix_block(blk)
    return n[0]


def build_program(split_waits=True, debug=False):
    nc = bacc_mod.Bacc()

    # ---- DRAM I/O ----
    xT_d = nc.dram_tensor("xT", [D, S], FP32, kind="ExternalInput")
    wq_d = nc.dram_tensor("wq_t", [D, HPD * DK], FP32, kind="ExternalInput")
    wk_d = nc.dram_tensor("wk_t", [D, HPD * DK], FP32, kind="ExternalInput")
    wv_d = nc.dram_tensor("wv_t", [D, HPD * DK], FP32, kind="ExternalInput")
    bq_d = nc.dram_tensor("bq", [1, HPD * DK], FP32, kind="ExternalInput")
    bk_d = nc.dram_tensor("bk_pair", [128, 2], FP32, kind="ExternalInput")
    bv_d = nc.dram_tensor("bv_pair", [128, 2], FP32, kind="ExternalInput")
    wc_d = nc.dram_tensor("wconv_t2", [128, K * DK], FP32, kind="ExternalInput")
    cb_d = nc.dram_tensor("conv_b2", [128, 1], FP32, kind="ExternalInput")
    wo_d = nc.dram_tensor("wo_t", [HPD * DK, D], FP32, kind="ExternalInput")
    y_d = nc.dram_tensor("y", [S, D], FP32, kind="ExternalOutput")
    utab_d = [
        nc.dram_tensor(f"utab{h}", [S * K, DK], FP32,
                       kind=("ExternalOutput" if debug else "Internal"))
        for h in range(HPD)
    ]
    idxd_d = [nc.dram_tensor(f"idxd{h}", [128, K * NCHUNK], U16, kind="Internal")
              for h in range(HPD)]
    if debug:
        idx_dbg = [nc.dram_tensor(f"idxdbg{h}", [128, NCHUNK, K], U16,
                                  kind="ExternalOutput") for h in range(HPD)]
        acc_dbg = [nc.dram_tensor(f"accdbg{p}", [128, NCHUNK, 128], FP32,
                                  kind="ExternalOutput") for p in range(2)]
        qnT_dbg = [nc.dram_tensor(f"qnTdbg{p}", [128, S], FP32,
                                  kind="ExternalOutput") for p in range(2)]
        kT_dbg = [nc.dram_tensor(f"kTdbg{p}", [128, S], FP32,
                                 kind="ExternalOutput") for p in range(2)]

    with tile.TileContext(nc) as tc, ExitStack() as ctx:
        con = ctx.enter_context(tc.tile_pool(name="consts", bufs=1))
        persist = ctx.enter_context(tc.tile_pool(name="persist", bufs=1))

        # ---- constants ----
        ident = con.tile([128, 128], FP32)
        make_identity(nc, ident[:])
        id2 = con.tile([128, DK], FP32)   # id2[p, j] = (p % 64 == j)
        make_identity(nc, id2[0:DK, :])
        make_identity(nc, id2[DK:128, :])
        kkpat_i = con.tile([128, NCHUNK, K], I32)
        nc.gpsimd.iota(kkpat_i[:], pattern=[[0, NCHUNK], [1, K]], base=0,
                       channel_multiplier=0)
        kkpat = con.tile([128, NCHUNK, K], FP32)
        nc.vector.tensor_copy(kkpat[:], kkpat_i[:])

        # weights / biases to SBUF
        wq_sb = con.tile([128, 4, HPD * DK], FP32)
        nc.sync.dma_start(wq_sb[:], wq_d[:].rearrange("(c p) n -> p c n", p=128))
        wk_sb = con.tile([128, 4, HPD * DK], FP32)
        nc.sync.dma_start(wk_sb[:], wk_d[:].rearrange("(c p) n -> p c n", p=128))
        wv_sb = con.tile([128, 4, HPD * DK], FP32)
        nc.sync.dma_start(wv_sb[:], wv_d[:].rearrange("(c p) n -> p c n", p=128))
        wc_sb = con.tile([128, K * DK], FP32)      # wconv_t replicated on both halves
        nc.sync.dma_start(wc_sb[:], wc_d[:])
        wo_sb = con.tile([DK, HPD, D], FP32)
        nc.sync.dma_start(wo_sb[:], wo_d[:].rearrange("(h c) n -> c h n", h=HPD))
        bk_sb = con.tile([128, 2], FP32)
        nc.sync.dma_start(bk_sb[:], bk_d[:])
        bv_sb = con.tile([128, 2], FP32)
        nc.sync.dma_start(bv_sb[:], bv_d[:])
        cb_sb = con.tile([128, 1], FP32)
        nc.sync.dma_start(cb_sb[:], cb_d[:])
        bq_sb = con.tile([128, HPD * DK], FP32)
        nc.sync.dma_start(bq_sb[:], bq_d[:].partition_broadcast(128))

        # persistent pair-stacked tensors: rows 0:64 = head 2p, 64:128 = head 2p+1
        qnT = [persist.tile([128, S], FP32, tag=f"qnT{p}", name=f"qnT{p}")
               for p in range(2)]
        kT = [persist.tile([128, S], FP32, tag=f"kT{p}", name=f"kT{p}")
              for p in range(2)]
        IDXR = [persist.tile([128, NCHUNK, K], U16, tag=f"idxr{h}", name=f"idxr{h}")
                for h in range(HPD)]
        idxu = [persist.tile([128, K, NCHUNK], U16, tag=f"idxu{h}", name=f"idxu{h}")
                for h in range(HPD)]

        with tc.tile_pool(name="proj", bufs=2) as proj, \
             tc.tile_pool(name="projv", bufs=1) as projv, \
             tc.tile_pool(name="psA", bufs=2, space=bass.MemorySpace.PSUM) as psA:
            xT_sb = projv.tile([128, 4, S], FP32, tag="xT")
            nc.sync.dma_start(xT_sb[:], xT_d[:].rearrange("(c p) n -> p c n", p=128))
            vT = [projv.tile([128, S], FP32, tag=f"vT{p}", name=f"vT{p}")
                  for p in range(2)]

            # ---- q natural + bias + normalize + pair-transpose to qnT ----
            for cch in range(NCHUNK):
                qp = psA.tile([128, HPD * DK], FP32, tag="qproj")
                for kc in range(4):
                    nc.tensor.matmul(
                        qp[:], xT_sb[:, kc, cch * 128:(cch + 1) * 128],
                        wq_sb[:, kc, :], start=(kc == 0), stop=(kc == 3),
                    )
                qsb = proj.tile([128, HPD * DK], FP32, tag="qsb")
                nc.vector.tensor_add(qsb[:], qp[:], bq_sb[:])
                sq = proj.tile([128, HPD * DK], FP32, tag="sq")
                nc.vector.tensor_mul(sq[:], qsb[:], qsb[:])
                ssq = proj.tile([128, HPD], FP32, tag="ssq")
                nc.vector.reduce_sum(ssq[:], sq[:].rearrange("p (h c) -> p h c", h=HPD),
                                     axis=mybir.AxisListType.X)
                nrm = proj.tile([128, HPD], FP32, tag="nrm")
                nc.scalar.sqrt(nrm[:], ssq[:])
                nc.vector.tensor_scalar_max(nrm[:], nrm[:], 1e-12)
                rinv = proj.tile([128, HPD], FP32, tag="rinv")
                nc.vector.reciprocal(rinv[:], nrm[:])
                for h in range(HPD):
                    nc.vector.tensor_scalar_mul(
                        qsb[:, h * DK:(h + 1) * DK], qsb[:, h * DK:(h + 1) * DK],
                        rinv[:, h:h + 1])
                for p in range(2):
                    tp = psA.tile([128, 128], FP32, tag="qtp")
                    nc.tensor.transpose(tp[:], qsb[:, p * 128:(p + 1) * 128], ident[:])
                    if p == 0:
                        nc.scalar.copy(qnT[p][:, cch * 128:(cch + 1) * 128], tp[:])
                    else:
                        nc.vector.tensor_copy(qnT[p][:, cch * 128:(cch + 1) * 128], tp[:])

            # ---- kT / vT pair-stacked + bias ----
            for p in range(2):
                for s4 in range(4):
                    for (dst, w_sb, b_sb) in ((kT, wk_sb, bk_sb), (vT, wv_sb, bv_sb)):
                        kp = psA.tile([128, 512], FP32, tag="kproj")
                        for kc in range(4):
                            nc.tensor.matmul(
                                kp[:],
                                w_sb[:, kc, p * 128:(p + 1) * 128],
                                xT_sb[:, kc, s4 * 512:(s4 + 1) * 512],
                                start=(kc == 0), stop=(kc == 3),
                            )
                        nc.vector.tensor_scalar_add(
                            dst[p][:, s4 * 512:(s4 + 1) * 512], kp[:],
                            b_sb[:, p:p + 1])

            # ---- U tables: rows (s, kk) of u_kk^T -> DRAM [S*K, DK]; one DMA/head
            for h in range(HPD):
                p, q = h // 2, h % 2
                usb = projv.tile([128, NCHUNK, K * DK], FP32, tag="usb", bufs=1,
                                 name="usb")
                for cch in range(NCHUNK):
                    upA = psA.tile([128, 288], FP32, tag="uprojA", bufs=1)
                    upB = psA.tile([128, 288], FP32, tag="uprojB", bufs=1)
                    lhs = vT[p][q * DK:(q + 1) * DK, cch * 128:(cch + 1) * 128]
                    nc.tensor.matmul(upA[:], lhs, wc_sb[q * DK:(q + 1) * DK, 0:288],
                                     start=True, stop=True)
                    nc.tensor.matmul(upB[:], lhs, wc_sb[q * DK:(q + 1) * DK, 288:576],
                                     start=True, stop=True)
                    nc.scalar.copy(usb[:, cch, 0:288], upA[:])
                    nc.scalar.copy(usb[:, cch, 288:576], upB[:])
                nc.sync.dma_start(
                    utab_d[h][:].rearrange("(cc p n) c -> p cc (n c)", p=128, n=K),
                    usb[:])

        # ---- per-head similarity + top-9 ----
        with tc.tile_pool(name="simpool", bufs=2, space=bass.MemorySpace.PSUM) as psS, \
             tc.tile_pool(name="topk", bufs=2) as tkp:
            for h in range(HPD):
                p, q = h // 2, h % 2
                scr9 = persist.tile([128, NCHUNK, 8], U16, tag="scr9", name="scr9")
                for cch in range(NCHUNK):
                    sim = psS.tile([128, S], FP32, tag="sim")
                    for jc in range(4):
                        nc.tensor.matmul(
                            sim[:, jc * 512:(jc + 1) * 512],
                            kT[p][q * DK:(q + 1) * DK, cch * 128:(cch + 1) * 128],
                            qnT[p][q * DK:(q + 1) * DK, jc * 512:(jc + 1) * 512],
                            start=True, stop=True,
                        )
                    cands = tkp.tile([128, NSEG * 8], FP32, tag="cands")
                    for sg in range(NSEG):
                        nc.vector.max(cands[:, sg * 8:(sg + 1) * 8],
                                      sim[:, sg * SEG:(sg + 1) * SEG])
                    g8 = tkp.tile([128, 8], FP32, tag="g8")
                    nc.vector.max(g8[:], cands[:])
                    c2 = tkp.tile([128, NSEG * 8], FP32, tag="c2")
                    nc.vector.match_replace(c2[:], g8[:], cands[:], NEG)
                    h8 = tkp.tile([128, 8], FP32, tag="h8")
                    nc.vector.max(h8[:], c2[:])
                    nc.vector.max_index(IDXR[h][:, cch, 0:8], g8[:], sim[:])
                    nc.vector.max_index(scr9[:, cch, :], h8[:], sim[:])
                nc.vector.tensor_copy(IDXR[h][:, :, 8], scr9[:, :, 0])
                idxf = tkp.tile([128, NCHUNK, K], FP32, tag="idxf")
                nc.vector.tensor_copy(idxf[:], IDXR[h][:])
                nc.vector.tensor_scalar(idxf[:], idxf[:], float(K), None,
                                        op0=mybir.AluOpType.mult)
                nc.vector.tensor_add(idxf[:], idxf[:], kkpat[:])
                nc.vector.tensor_copy(idxu[h][:].rearrange("p k g -> p g k"), idxf[:])
                nc.sync.dma_start(idxd_d[h][:], idxu[h][:].rearrange("p k g -> p (k g)"))

        # ---- gather + accumulate, transposes, scramble, Wo ----
        with tc.tile_pool(name="late", bufs=1) as late, \
             tc.tile_pool(name="psB", bufs=2, space=bass.MemorySpace.PSUM) as psB:
            acc = [late.tile([128, NCHUNK, 128], FP32, tag=f"acc{p}", name=f"acc{p}")
                   for p in range(2)]
            NIDX = S * K  # 18432
            for h in range(HPD):
                p, q = h // 2, h % 2
                # wrapped-16 idx layout for dma_gather, replicated to all cores
                idx16 = late.tile([128, NIDX // 16], U16, tag="idx16", bufs=1,
                                  name="idx16")
                idview = idxd_d[h][:].rearrange("(w pp) (k g) -> pp k g w", w=8, k=K)
                for grp in range(8):
                    nc.sync.dma_start(idx16[grp * 16:(grp + 1) * 16, :].rearrange(
                        "pp (k g w) -> pp k g w", k=K, w=8), idview)
                gball = late.tile([128, K, NCHUNK, DK], FP32, tag="gball", bufs=1,
                                  name="gball")
                gflat = gball[:].rearrange("p k g c -> p (k g) c")
                for half in range(2):
                    nc.gpsimd.dma_gather(
                        out_ap=gflat[:, half * 72:(half + 1) * 72, :],
                        in_ap=utab_d[h][:],
                        idxs_ap=idx16[:, half * 576:(half + 1) * 576].bitcast(I16),
                        num_idxs=NIDX // 2,
                        num_idxs_reg=NIDX // 2,
                        elem_size=DK,
                        single_packet=False,
                    )
                gb = [gball[:, kk, :, :] for kk in range(K)]
                # tree-sum the 9 slices into acc (DVE + GPSIMD split, in place)
                nc.vector.tensor_add(gb[0], gb[0], gb[1])
                nc.gpsimd.tensor_add(gb[2], gb[2], gb[3])
                nc.vector.tensor_add(gb[4], gb[4], gb[5])
                nc.gpsimd.tensor_add(gb[6], gb[6], gb[7])
                nc.vector.tensor_add(gb[0], gb[0], gb[4])
                nc.gpsimd.tensor_add(gb[2], gb[2], gb[6])
                nc.vector.tensor_add(gb[0], gb[0], gb[8])
                nc.vector.tensor_add(acc[p][:, :, q * DK:(q + 1) * DK],
                                     gb[0], gb[2])
            outT = [late.tile([128, S], FP32, tag=f"outT{p}", name=f"outT{p}")
                    for p in range(2)]
            for p in range(2):
                for cch in range(NCHUNK):
                    tp = psB.tile([128, 128], FP32, tag="otp")
                    nc.tensor.transpose(tp[:], acc[p][:, cch, :], ident[:])
                    if p == 0:
                        nc.scalar.add(outT[p][:, cch * 128:(cch + 1) * 128], tp[:],
                                      cb_sb[:, 0:1])
                    else:
                        nc.vector.tensor_scalar_add(
                            outT[p][:, cch * 128:(cch + 1) * 128], tp[:], cb_sb[:, 0:1])
            # scramble: AT_h[c2, 64a+r] = outT_pair[r(+64q), 32*c2+a]
            AT = [late.tile([DK, S], FP32, tag=f"AT{h}", name=f"AT{h}")
                  for h in range(HPD)]
            for h in range(HPD):
                p, q = h // 2, h % 2
                for a in range(32):
                    tp = psB.tile([DK, DK], FP32, tag="atp")
                    nc.tensor.transpose(tp[:], outT[p][q * DK:(q + 1) * DK, a:S:32],
                                        id2[q * DK:(q + 1) * DK, :])
                    if h % 2 == 0:
                        nc.scalar.copy(AT[h][:, a * DK:(a + 1) * DK], tp[:])
                    else:
                        nc.vector.tensor_copy(AT[h][:, a * DK:(a + 1) * DK], tp[:])
            # Wo partial projection (accumulate over 4 heads, K=64 each)
            for cch in range(NCHUNK):
                yp = psB.tile([128, D], FP32, tag="yp")
                for h in range(HPD):
                    nc.tensor.matmul(yp[:], AT[h][:, cch * 128:(cch + 1) * 128],
                                     wo_sb[:, h, :], start=(h == 0), stop=(h == HPD - 1))
                yb = late.tile([128, D], FP32, tag="ybounce", bufs=2, name="yb")
                nc.scalar.copy(yb[:], yp[:])
                nc.sync.dma_start(y_d[cch * 128:(cch + 1) * 128, :], yb[:])
            if debug:
                for h in range(HPD):
                    nc.sync.dma_start(idx_dbg[h][:], IDXR[h][:])
                for p in range(2):
                    nc.sync.dma_start(acc_dbg[p][:], acc[p][:])
                    nc.sync.dma_start(qnT_dbg[p][:], qnT[p][:])
                    nc.sync.dma_start(kT_dbg[p][:], kT[p][:])

    nc.compile()
    if split_waits:
        _split_multi_waits(nc)
    return nc


_CACHED = {}


def _get_program():
    if "nc" not in _CACHED:
        _CACHED["nc"] = build_program()
    return _CACHED["nc"]


def make_in_maps(x, Wq, bq, Wk, bk, Wv, bv, Wo, cw, cb):
    wconv_t = np.concatenate([cw[:, :, kk].T for kk in range(K)], axis=1)  # [64, 576]
    wconv_t2 = np.concatenate([wconv_t, wconv_t], axis=0)                  # [128, 576]
    cb2 = np.concatenate([cb, cb])[:, None]                                # [128, 1]
    in_maps = []
    for d in range(8):
        b, g = d // 2, d % 2
        rows = slice(g * HPD * DK, (g + 1) * HPD * DK)
        bk_l, bv_l = bk[rows], bv[rows]
        bk_pair = np.stack([bk_l[0:128], bk_l[128:256]], axis=1)
        bv_pair = np.stack([bv_l[0:128], bv_l[128:256]], axis=1)
        in_maps.append({
            "xT": np.ascontiguousarray(x[b].T),
            "wq_t": np.ascontiguousarray(Wq[rows].T),
            "wk_t": np.ascontiguousarray(Wk[rows].T),
            "wv_t": np.ascontiguousarray(Wv[rows].T),
            "bq": np.ascontiguousarray(bq[rows][None, :]),
            "bk_pair": np.ascontiguousarray(bk_pair),
            "bv_pair": np.ascontiguousarray(bv_pair),
            "wconv_t2": np.ascontiguousarray(wconv_t2),
            "conv_b2": np.ascontiguousarray(cb2),
            "wo_t": np.ascontiguousarray(Wo[:, rows].T),
        })
    return in_maps


def kernel(**inputs):
    from concourse.bass_utils import run_bass_kernel_spmd

    x = np.asarray(inputs["x"], np.float32)
    Wq = np.asarray(inputs["Wq_w"], np.float32)
    bq = np.asarray(inputs["Wq_b"], np.float32)
    Wk = np.asarray(inputs["Wk_w"], np.float32)
    bk = np.asarray(inputs["Wk_b"], np.float32)
    Wv = np.asarray(inputs["Wv_w"], np.float32)
    bv = np.asarray(inputs["Wv_b"], np.float32)
    Wo = np.asarray(inputs["Wo_w"], np.float32)
    bo = np.asarray(inputs["Wo_b"], np.float32)
    cw = np.asarray(inputs["conv_w"], np.float32)
    cb = np.asarray(inputs["conv_b"], np.float32)

    nc = _get_program()
    in_maps = make_in_maps(x, Wq, bq, Wk, bk, Wv, bv, Wo, cw, cb)
    res = run_bass_kernel_spmd(nc, in_maps, core_ids=list(range(8)))
    y = np.zeros((B, S, D), np.float32)
    for b in range(B):
        y[b] = res.results[2 * b]["y"] + res.results[2 * b + 1]["y"] + bo[None, :]
    return y


if __name__ == "__main__":
    nc = build_program()
    print("program built ok")


# revision 26
# speedup vs baseline: 1.7414x; 1.7414x over previous
"""Trainium2 Bass kernel for nn_MultiHeadConvNNAttention.

Sharding: 8 cores; core d handles batch b = d//2 and head-group g = d%2
(4 heads of H=8 each). Per core: q/k/v projections (fp32 on PE), per-head
SxS similarity (k^T @ qn, fp32), exact top-9 per row via segmented DVE
max8/max_index, neighbor-conv as 9 accumulating indirect-DMA gathers from a
precomputed u-table in DRAM, the torch-view output scramble via PE
transposes, and a partial Wo projection. Host sums the two partial y's per
batch and adds Wo_b.

Self-contained: hardcodes all shapes; does not read /root/problem files.
"""
import sys
import numpy as np

sys.path.insert(0, "/opt/trn_rl_repo")

import bass_rust
import concourse.bass as bass
import concourse.bacc as bacc_mod
import concourse.mybir as mybir
import concourse.tile as tile
from concourse.masks import make_identity
from contextlib import ExitStack

B, S, D, H, K = 4, 2048, 512, 8, 9
DK = D // H          # 64
HPD = H // 2         # heads per device = 4
NCHUNK = S // 128    # 16 i-chunks
NSEG = 8             # row segments for seg-max8
SEG = S // NSEG      # 256
FP32 = mybir.dt.float32
U16 = mybir.dt.uint16
I16 = mybir.dt.int16
U32 = mybir.dt.uint32
I32 = mybir.dt.int32
NEG = -3.0e38


def _split_multi_waits(nc):
    """This walrus build supports at most one sem-wait per instruction;
    split extras onto preceding same-engine drain carriers."""
    n = [0]

    def fix_block(blk):
        insts = blk.instructions
        out = []
        changed = False
        for i in insts:
            si = i.sync_info
            ow = list(si.on_wait) if si is not None and si.on_wait is not None else []
            if len(ow) > 1:
                changed = True
                for w in ow[:-1]:
                    n[0] += 1
                    c = mybir.InstDrain(name=f"wsplit_{n[0]}", ins=[], outs=[])
                    c.engine = i.engine
                    c.sync_info = bass_rust.SyncInfo(on_wait=[w], on_update=[])
                    out.append(c)
                i.sync_info = bass_rust.SyncInfo(
                    on_wait=[ow[-1]], on_update=list(si.on_update or []))
            out.append(i)
        if changed:
            blk.instructions = out

    for f in nc.m.functions:
        for blk in f.blocks:
            fix_block(blk)
    return n[0]


def build_program(split_waits=True, debug=False):
    nc = bacc_mod.Bacc()

    # ---- DRAM I/O ----
    xT_d = nc.dram_tensor("xT", [D, S], FP32, kind="ExternalInput")
    wq_d = nc.dram_tensor("wq_t", [D, HPD * DK], FP32, kind="ExternalInput")
    wk_d = nc.dram_tensor("wk_t", [D, HPD * DK], FP32, kind="ExternalInput")
    wv_d = nc.dram_tensor("wv_t", [D, HPD * DK], FP32, kind="ExternalInput")
    bq_d = nc.dram_tensor("bq", [1, HPD * DK], FP32, kind="ExternalInput")
    bk_d = nc.dram_tensor("bk_pair", [128, 2], FP32, kind="ExternalInput")
    bv_d = nc.dram_tensor("bv_pair", [128, 2], FP32, kind="ExternalInput")
    wc_d = nc.dram_tensor("wconv_t2", [128, K * DK], FP32, kind="ExternalInput")
    cb_d = nc.dram_tensor("conv_b2", [128, 1], FP32, kind="ExternalInput")
    wo_d = nc.dram_tensor("wo_t", [HPD * DK, D], FP32, kind="ExternalInput")
    y_d = nc.dram_tensor("y", [S, D], FP32, kind="ExternalOutput")
    utab_d = [
        nc.dram_tensor(f"utab{h}", [S * K, DK], FP32,
                       kind=("ExternalOutput" if debug else "Internal"))
        for h in range(HPD)
    ]
    idxd_d = [nc.dram_tensor(f"idxd{h}", [128, K * NCHUNK], U16, kind="Internal")
              for h in range(HPD)]
    if debug:
        idx_dbg = [nc.dram_tensor(f"idxdbg{h}", [128, NCHUNK, K], U16,
                                  kind="ExternalOutput") for h in range(HPD)]
        acc_dbg = [nc.dram_tensor(f"accdbg{p}", [128, NCHUNK, 128], FP32,
                                  kind="ExternalOutput") for p in range(2)]
        qnT_dbg = [nc.dram_tensor(f"qnTdbg{p}", [128, S], FP32,
                                  kind="ExternalOutput") for p in range(2)]
        kT_dbg = [nc.dram_tensor(f"kTdbg{p}", [128, S], FP32,
                                 kind="ExternalOutput") for p in range(2)]

    with tile.TileContext(nc) as tc, ExitStack() as ctx:
        con = ctx.enter_context(tc.tile_pool(name="consts", bufs=1))
        persist = ctx.enter_context(tc.tile_pool(name="persist", bufs=1))

        # ---- constants ----
        ident = con.tile([128, 128], FP32)
        make_identity(nc, ident[:])
        id2 = con.tile([128, DK], FP32)   # id2[p, j] = (p % 64 == j)
        make_identity(nc, id2[0:DK, :])
        make_identity(nc, id2[DK:128, :])
        kkpat_i = con.tile([128, NCHUNK, K], I32)
        nc.gpsimd.iota(kkpat_i[:], pattern=[[0, NCHUNK], [1, K]], base=0,
                       channel_multiplier=0)
        kkpat = con.tile([128, NCHUNK, K], FP32)
        nc.vector.tensor_copy(kkpat[:], kkpat_i[:])

        # weights / biases to SBUF
        wq_sb = con.tile([128, 4, HPD * DK], FP32)
        nc.sync.dma_start(wq_sb[:], wq_d[:].rearrange("(c p) n -> p c n", p=128))
        wk_sb = con.tile([128, 4, HPD * DK], FP32)
        nc.sync.dma_start(wk_sb[:], wk_d[:].rearrange("(c p) n -> p c n", p=128))
        wv_sb = con.tile([128, 4, HPD * DK], FP32)
        nc.sync.dma_start(wv_sb[:], wv_d[:].rearrange("(c p) n -> p c n", p=128))
        wc_sb = con.tile([128, K * DK], FP32)      # wconv_t replicated on both halves
        nc.sync.dma_start(wc_sb[:], wc_d[:])
        wo_sb = con.tile([DK, HPD, D], FP32)
        nc.sync.dma_start(wo_sb[:], wo_d[:].rearrange("(h c) n -> c h n", h=HPD))
        bk_sb = con.tile([128, 2], FP32)
        nc.sync.dma_start(bk_sb[:], bk_d[:])
        bv_sb = con.tile([128, 2], FP32)
        nc.sync.dma_start(bv_sb[:], bv_d[:])
        cb_sb = con.tile([128, 1], FP32)
        nc.sync.dma_start(cb_sb[:], cb_d[:])
        bq_sb = con.tile([128, HPD * DK], FP32)
        nc.sync.dma_start(bq_sb[:], bq_d[:].partition_broadcast(128))

        # persistent pair-stacked tensors: rows 0:64 = head 2p, 64:128 = head 2p+1
        qnT = [persist.tile([128, S], FP32, tag=f"qnT{p}", name=f"qnT{p}")
               for p in range(2)]
        kT = [persist.tile([128, S], FP32, tag=f"kT{p}", name=f"kT{p}")
              for p in range(2)]
        IDXR = [persist.tile([128, NCHUNK, K], U16, tag=f"idxr{h}", name=f"idxr{h}")
                for h in range(HPD)]
        idxu = [persist.tile([128, K, NCHUNK], U16, tag=f"idxu{h}", name=f"idxu{h}")
                for h in range(HPD)]

        with tc.tile_pool(name="proj", bufs=2) as proj, \
             tc.tile_pool(name="projv", bufs=1) as projv, \
             tc.tile_pool(name="psA", bufs=2, space=bass.MemorySpace.PSUM) as psA:
            xT_sb = projv.tile([128, 4, S], FP32, tag="xT")
            nc.sync.dma_start(xT_sb[:], xT_d[:].rearrange("(c p) n -> p c n", p=128))
            vT = [projv.tile([128, S], FP32, tag=f"vT{p}", name=f"vT{p}")
                  for p in range(2)]

            # ---- q natural + bias + normalize + pair-transpose to qnT ----
            for cch in range(NCHUNK):
                qp = psA.tile([128, HPD * DK], FP32, tag="qproj")
                for kc in range(4):
                    nc.tensor.matmul(
                        qp[:], xT_sb[:, kc, cch * 128:(cch + 1) * 128],
                        wq_sb[:, kc, :], start=(kc == 0), stop=(kc == 3),
                    )
                qsb = proj.tile([128, HPD * DK], FP32, tag="qsb")
                nc.vector.tensor_add(qsb[:], qp[:], bq_sb[:])
                sq = proj.tile([128, HPD * DK], FP32, tag="sq")
                nc.vector.tensor_mul(sq[:], qsb[:], qsb[:])
                ssq = proj.tile([128, HPD], FP32, tag="ssq")
                nc.vector.reduce_sum(ssq[:], sq[:].rearrange("p (h c) -> p h c", h=HPD),
                                     axis=mybir.AxisListType.X)
                nrm = proj.tile([128, HPD], FP32, tag="nrm")
                nc.scalar.sqrt(nrm[:], ssq[:])
                nc.vector.tensor_scalar_max(nrm[:], nrm[:], 1e-12)
                rinv = proj.tile([128, HPD], FP32, tag="rinv")
                nc.vector.reciprocal(rinv[:], nrm[:])
                for h in range(HPD):
                    nc.vector.tensor_scalar_mul(
                        qsb[:, h * DK:(h + 1) * DK], qsb[:, h * DK:(h + 1) * DK],
                        rinv[:, h:h + 1])
                for p in range(2):
                    tp = psA.tile([128, 128], FP32, tag="qtp")
                    nc.tensor.transpose(tp[:], qsb[:, p * 128:(p + 1) * 128], ident[:])
                    if p == 0:
                        nc.scalar.copy(qnT[p][:, cch * 128:(cch + 1) * 128], tp[:])
                    else:
                        nc.vector.tensor_copy(qnT[p][:, cch * 128:(cch + 1) * 128], tp[:])

            # ---- kT / vT pair-stacked + bias ----
            for p in range(2):
                for s4 in range(4):
                    for (dst, w_sb, b_sb) in ((kT, wk_sb, bk_sb), (vT, wv_sb, bv_sb)):
                        kp = psA.tile([128, 512], FP32, tag="kproj")
                        for kc in range(4):
                            nc.tensor.matmul(
                                kp[:],
                                w_sb[:, kc, p * 128:(p + 1) * 128],
                                xT_sb[:, kc, s4 * 512:(s4 + 1) * 512],
                                start=(kc == 0), stop=(kc == 3),
                            )
                        nc.vector.tensor_scalar_add(
                            dst[p][:, s4 * 512:(s4 + 1) * 512], kp[:],
                            b_sb[:, p:p + 1])

            # ---- U tables: rows (s, kk) of u_kk^T -> DRAM [S*K, DK]; one DMA/head
            for h in range(HPD):
                p, q = h // 2, h % 2
                usb = projv.tile([128, NCHUNK, K * DK], FP32, tag="usb", bufs=1,
                                 name="usb")
                for cch in range(NCHUNK):
                    upA = psA.tile([128, 288], FP32, tag="uprojA", bufs=1)
                    upB = psA.tile([128, 288], FP32, tag="uprojB", bufs=1)
                    lhs = vT[p][q * DK:(q + 1) * DK, cch * 128:(cch + 1) * 128]
                    nc.tensor.matmul(upA[:], lhs, wc_sb[q * DK:(q + 1) * DK, 0:288],
                                     start=True, stop=True)
                    nc.tensor.matmul(upB[:], lhs, wc_sb[q * DK:(q + 1) * DK, 288:576],
                                     start=True, stop=True)
                    nc.scalar.copy(usb[:, cch, 0:288], upA[:])
                    nc.scalar.copy(usb[:, cch, 288:576], upB[:])
                nc.sync.dma_start(
                    utab_d[h][:].rearrange("(cc p n) c -> p cc (n c)", p=128, n=K),
                    usb[:])

        # ---- per-head similarity + top-9 ----
        with tc.tile_pool(name="simpool", bufs=2, space=bass.MemorySpace.PSUM) as psS, \
             tc.tile_pool(name="topk", bufs=2) as tkp:
            for h in range(HPD):
                p, q = h // 2, h % 2
                scr9 = persist.tile([128, NCHUNK, 8], U16, tag="scr9", name="scr9")
                for cch in range(NCHUNK):
                    sim = psS.tile([128, S], FP32, tag="sim")
                    for jc in range(4):
                        nc.tensor.matmul(
                            sim[:, jc * 512:(jc + 1) * 512],
                            kT[p][q * DK:(q + 1) * DK, cch * 128:(cch + 1) * 128],
                            qnT[p][q * DK:(q + 1) * DK, jc * 512:(jc + 1) * 512],
                            start=True, stop=True,
                        )
                    cands = tkp.tile([128, NSEG * 8], FP32, tag="cands")
                    for sg in range(NSEG):
                        nc.vector.max(cands[:, sg * 8:(sg + 1) * 8],
                                      sim[:, sg * SEG:(sg + 1) * SEG])
                    g8 = tkp.tile([128, 8], FP32, tag="g8")
                    nc.vector.max(g8[:], cands[:])
                    c2 = tkp.tile([128, NSEG * 8], FP32, tag="c2")
                    nc.vector.match_replace(c2[:], g8[:], cands[:], NEG)
                    h8 = tkp.tile([128, 8], FP32, tag="h8")
                    nc.vector.max(h8[:], c2[:])
                    nc.vector.max_index(IDXR[h][:, cch, 0:8], g8[:], sim[:])
                    nc.vector.max_index(scr9[:, cch, :], h8[:], sim[:])
                nc.vector.tensor_copy(IDXR[h][:, :, 8], scr9[:, :, 0])
                idxf = tkp.tile([128, NCHUNK, K], FP32, tag="idxf")
                nc.vector.tensor_copy(idxf[:], IDXR[h][:])
                nc.vector.tensor_scalar(idxf[:], idxf[:], float(K), None,
                                        op0=mybir.AluOpType.mult)
                nc.vector.tensor_add(idxf[:], idxf[:], kkpat[:])
                nc.vector.tensor_copy(idxu[h][:].rearrange("p k g -> p g k"), idxf[:])
                nc.sync.dma_start(idxd_d[h][:], idxu[h][:].rearrange("p k g -> p (k g)"))

        # ---- gather + accumulate, transposes, scramble, Wo ----
        with tc.tile_pool(name="late", bufs=1) as late, \
             tc.tile_pool(name="psB", bufs=2, space=bass.MemorySpace.PSUM) as psB:
            acc = [late.tile([128, NCHUNK, 128], FP32, tag=f"acc{p}", name=f"acc{p}")
                   for p in range(2)]
            NIDX = S * K  # 18432
            for h in range(HPD):
                p, q = h // 2, h % 2
                # wrapped-16 idx layout for dma_gather, replicated to all cores
                idx16 = late.tile([128, NIDX // 16], U16, tag="idx16", bufs=1,
                                  name="idx16")
                staged = late.tile([16, 8, K * NCHUNK], U16, tag="staged", bufs=1,
                                   name="staged")
                nc.sync.dma_start(
                    staged[:], idxd_d[h][:].rearrange("(w pp) f -> pp w f", w=8))
                nc.vector.tensor_copy(
                    idx16[0:16, :].rearrange("pp (k g w) -> pp k g w", k=K, w=8),
                    staged[:].rearrange("pp w (k g) -> pp k g w", k=K))
                nc.sync.dma_start(idx16[16:32, :], idx16[0:16, :])
                nc.sync.dma_start(idx16[32:64, :], idx16[0:32, :])
                nc.sync.dma_start(idx16[64:128, :], idx16[0:64, :])
                gball = late.tile([128, K, NCHUNK, DK], FP32, tag="gball", bufs=1,
                                  name="gball")
                gflat = gball[:].rearrange("p k g c -> p (k g) c")
                NG = 18
                step = NIDX // NG
                for j in range(NG):
                    nc.gpsimd.dma_gather(
                        out_ap=gflat[:, j * (step // 128):(j + 1) * (step // 128), :],
                        in_ap=utab_d[h][:],
                        idxs_ap=idx16[:, j * (step // 16):(j + 1) * (step // 16)].bitcast(I16),
                        num_idxs=step,
                        num_idxs_reg=step,
                        elem_size=DK,
                    )
                gb = [gball[:, kk, :, :] for kk in range(K)]
                # tree-sum the 9 slices into acc (DVE + GPSIMD split, in place)
                nc.vector.tensor_add(gb[0], gb[0], gb[1])
                nc.gpsimd.tensor_add(gb[2], gb[2], gb[3])
                nc.vector.tensor_add(gb[4], gb[4], gb[5])
                nc.gpsimd.tensor_add(gb[6], gb[6], gb[7])
                nc.vector.tensor_add(gb[0], gb[0], gb[4])
                nc.gpsimd.tensor_add(gb[2], gb[2], gb[6])
                nc.vector.tensor_add(gb[0], gb[0], gb[8])
                nc.vector.tensor_add(acc[p][:, :, q * DK:(q + 1) * DK],
                                     gb[0], gb[2])
            outT = [late.tile([128, S], FP32, tag=f"outT{p}", name=f"outT{p}")
                    for p in range(2)]
            for p in range(2):
                for cch in range(NCHUNK):
                    tp = psB.tile([128, 128], FP32, tag="otp")
                    nc.tensor.transpose(tp[:], acc[p][:, cch, :], ident[:])
                    if p == 0:
                        nc.scalar.add(outT[p][:, cch * 128:(cch + 1) * 128], tp[:],
                                      cb_sb[:, 0:1])
                    else:
                        nc.vector.tensor_scalar_add(
                            outT[p][:, cch * 128:(cch + 1) * 128], tp[:], cb_sb[:, 0:1])
            # scramble: AT_h[c2, 64a+r] = outT_pair[r(+64q), 32*c2+a]
            AT = [late.tile([DK, S], FP32, tag=f"AT{h}", name=f"AT{h}")
                  for h in range(HPD)]
            for h in range(HPD):
                p, q = h // 2, h % 2
                for a in range(32):
                    tp = psB.tile([DK, DK], FP32, tag="atp")
                    nc.tensor.transpose(tp[:], outT[p][q * DK:(q + 1) * DK, a:S:32],
                                        id2[q * DK:(q + 1) * DK, :])
                    if h % 2 == 0:
                        nc.scalar.copy(AT[h][:, a * DK:(a + 1) * DK], tp[:])
                    else:
                        nc.vector.tensor_copy(AT[h][:, a * DK:(a + 1) * DK], tp[:])
            # Wo partial projection (accumulate over 4 heads, K=64 each)
            for cch in range(NCHUNK):
                yp = psB.tile([128, D], FP32, tag="yp")
                for h in range(HPD):
                    nc.tensor.matmul(yp[:], AT[h][:, cch * 128:(cch + 1) * 128],
                                     wo_sb[:, h, :], start=(h == 0), stop=(h == HPD - 1))
                yb = late.tile([128, D], FP32, tag="ybounce", bufs=2, name="yb")
                nc.scalar.copy(yb[:], yp[:])
                nc.sync.dma_start(y_d[cch * 128:(cch + 1) * 128, :], yb[:])
            if debug:
                for h in range(HPD):
                    nc.sync.dma_start(idx_dbg[h][:], IDXR[h][:])
                for p in range(2):
                    nc.sync.dma_start(acc_dbg[p][:], acc[p][:])
                    nc.sync.dma_start(qnT_dbg[p][:], qnT[p][:])
                    nc.sync.dma_start(kT_dbg[p][:], kT[p][:])

    nc.compile()
    if split_waits:
        _split_multi_waits(nc)
    return nc


_CACHED = {}


def _get_program():
    if "nc" not in _CACHED:
        _CACHED["nc"] = build_program()
    return _CACHED["nc"]


def make_in_maps(x, Wq, bq, Wk, bk, Wv, bv, Wo, cw, cb):
    wconv_t = np.concatenate([cw[:, :, kk].T for kk in range(K)], axis=1)  # [64, 576]
    wconv_t2 = np.concatenate([wconv_t, wconv_t], axis=0)                  # [128, 576]
    cb2 = np.concatenate([cb, cb])[:, None]                                # [128, 1]
    in_maps = []
    for d in range(8):
        b, g = d // 2, d % 2
        rows = slice(g * HPD * DK, (g + 1) * HPD * DK)
        bk_l, bv_l = bk[rows], bv[rows]
        bk_pair = np.stack([bk_l[0:128], bk_l[128:256]], axis=1)
        bv_pair = np.stack([bv_l[0:128], bv_l[128:256]], axis=1)
        in_maps.append({
            "xT": np.ascontiguousarray(x[b].T),
            "wq_t": np.ascontiguousarray(Wq[rows].T),
            "wk_t": np.ascontiguousarray(Wk[rows].T),
            "wv_t": np.ascontiguousarray(Wv[rows].T),
            "bq": np.ascontiguousarray(bq[rows][None, :]),
            "bk_pair": np.ascontiguousarray(bk_pair),
            "bv_pair": np.ascontiguousarray(bv_pair),
            "wconv_t2": np.ascontiguousarray(wconv_t2),
            "conv_b2": np.ascontiguousarray(cb2),
            "wo_t": np.ascontiguousarray(Wo[:, rows].T),
        })
    return in_maps


def kernel(**inputs):
    from concourse.bass_utils import run_bass_kernel_spmd

    x = np.asarray(inputs["x"], np.float32)
    Wq = np.asarray(inputs["Wq_w"], np.float32)
    bq = np.asarray(inputs["Wq_b"], np.float32)
    Wk = np.asarray(inputs["Wk_w"], np.float32)
    bk = np.asarray(inputs["Wk_b"], np.float32)
    Wv = np.asarray(inputs["Wv_w"], np.float32)
    bv = np.asarray(inputs["Wv_b"], np.float32)
    Wo = np.asarray(inputs["Wo_w"], np.float32)
    bo = np.asarray(inputs["Wo_b"], np.float32)
    cw = np.asarray(inputs["conv_w"], np.float32)
    cb = np.asarray(inputs["conv_b"], np.float32)

    nc = _get_program()
    in_maps = make_in_maps(x, Wq, bq, Wk, bk, Wv, bv, Wo, cw, cb)
    res = run_bass_kernel_spmd(nc, in_maps, core_ids=list(range(8)))
    y = np.zeros((B, S, D), np.float32)
    for b in range(B):
        y[b] = res.results[2 * b]["y"] + res.results[2 * b + 1]["y"] + bo[None, :]
    return y


if __name__ == "__main__":
    nc = build_program()
    print("program built ok")


# revision 27
# speedup vs baseline: 1.7662x; 1.0142x over previous
"""Trainium2 Bass kernel for nn_MultiHeadConvNNAttention.

Sharding: 8 cores; core d handles batch b = d//2 and head-group g = d%2
(4 heads of H=8 each). Per core: q/k/v projections (fp32 on PE), per-head
SxS similarity (k^T @ qn, fp32), exact top-9 per row via segmented DVE
max8/max_index, neighbor-conv as 9 accumulating indirect-DMA gathers from a
precomputed u-table in DRAM, the torch-view output scramble via PE
transposes, and a partial Wo projection. Host sums the two partial y's per
batch and adds Wo_b.

Self-contained: hardcodes all shapes; does not read /root/problem files.
"""
import sys
import numpy as np

sys.path.insert(0, "/opt/trn_rl_repo")

import bass_rust
import concourse.bass as bass
import concourse.bacc as bacc_mod
import concourse.mybir as mybir
import concourse.tile as tile
from concourse.masks import make_identity
from contextlib import ExitStack

B, S, D, H, K = 4, 2048, 512, 8, 9
DK = D // H          # 64
HPD = H // 2         # heads per device = 4
NCHUNK = S // 128    # 16 i-chunks
NSEG = 8             # row segments for seg-max8
SEG = S // NSEG      # 256
FP32 = mybir.dt.float32
U16 = mybir.dt.uint16
I16 = mybir.dt.int16
U32 = mybir.dt.uint32
I32 = mybir.dt.int32
NEG = -3.0e38


def _split_multi_waits(nc):
    """This walrus build supports at most one sem-wait per instruction;
    split extras onto preceding same-engine drain carriers."""
    n = [0]

    def fix_block(blk):
        insts = blk.instructions
        out = []
        changed = False
        for i in insts:
            si = i.sync_info
            ow = list(si.on_wait) if si is not None and si.on_wait is not None else []
            if len(ow) > 1:
                changed = True
                for w in ow[:-1]:
                    n[0] += 1
                    c = mybir.InstDrain(name=f"wsplit_{n[0]}", ins=[], outs=[])
                    c.engine = i.engine
                    c.sync_info = bass_rust.SyncInfo(on_wait=[w], on_update=[])
                    out.append(c)
                i.sync_info = bass_rust.SyncInfo(
                    on_wait=[ow[-1]], on_update=list(si.on_update or []))
            out.append(i)
        if changed:
            blk.instructions = out

    for f in nc.m.functions:
        for blk in f.blocks:
            fix_block(blk)
    return n[0]


def build_program(split_waits=True, debug=False):
    nc = bacc_mod.Bacc()

    # ---- DRAM I/O ----
    xT_d = nc.dram_tensor("xT", [D, S], FP32, kind="ExternalInput")
    wq_d = nc.dram_tensor("wq_t", [D, HPD * DK], FP32, kind="ExternalInput")
    wk_d = nc.dram_tensor("wk_t", [D, HPD * DK], FP32, kind="ExternalInput")
    wv_d = nc.dram_tensor("wv_t", [D, HPD * DK], FP32, kind="ExternalInput")
    bq_d = nc.dram_tensor("bq", [1, HPD * DK], FP32, kind="ExternalInput")
    bk_d = nc.dram_tensor("bk_pair", [128, 2], FP32, kind="ExternalInput")
    bv_d = nc.dram_tensor("bv_pair", [128, 2], FP32, kind="ExternalInput")
    wc_d = nc.dram_tensor("wconv_t2", [128, K * DK], FP32, kind="ExternalInput")
    cb_d = nc.dram_tensor("conv_b2", [128, 1], FP32, kind="ExternalInput")
    wo_d = nc.dram_tensor("wo_t", [HPD * DK, D], FP32, kind="ExternalInput")
    y_d = nc.dram_tensor("y", [S, D], FP32, kind="ExternalOutput")
    utab_d = [
        nc.dram_tensor(f"utab{h}", [S * K, DK], FP32,
                       kind=("ExternalOutput" if debug else "Internal"))
        for h in range(HPD)
    ]
    idxd_d = [nc.dram_tensor(f"idxd{h}", [128, K * NCHUNK], U16, kind="Internal")
              for h in range(HPD)]
    if debug:
        idx_dbg = [nc.dram_tensor(f"idxdbg{h}", [128, NCHUNK, K], U16,
                                  kind="ExternalOutput") for h in range(HPD)]
        acc_dbg = [nc.dram_tensor(f"accdbg{p}", [128, NCHUNK, 128], FP32,
                                  kind="ExternalOutput") for p in range(2)]
        qnT_dbg = [nc.dram_tensor(f"qnTdbg{p}", [128, S], FP32,
                                  kind="ExternalOutput") for p in range(2)]
        kT_dbg = [nc.dram_tensor(f"kTdbg{p}", [128, S], FP32,
                                 kind="ExternalOutput") for p in range(2)]

    with tile.TileContext(nc) as tc, ExitStack() as ctx:
        con = ctx.enter_context(tc.tile_pool(name="consts", bufs=1))
        persist = ctx.enter_context(tc.tile_pool(name="persist", bufs=1))

        # ---- constants ----
        ident = con.tile([128, 128], FP32)
        make_identity(nc, ident[:])
        id2 = con.tile([128, DK], FP32)   # id2[p, j] = (p % 64 == j)
        make_identity(nc, id2[0:DK, :])
        make_identity(nc, id2[DK:128, :])
        kkpat_i = con.tile([128, NCHUNK, K], I32)
        nc.gpsimd.iota(kkpat_i[:], pattern=[[0, NCHUNK], [1, K]], base=0,
                       channel_multiplier=0)
        kkpat = con.tile([128, NCHUNK, K], FP32)
        nc.vector.tensor_copy(kkpat[:], kkpat_i[:])

        # weights / biases to SBUF
        wq_sb = con.tile([128, 4, HPD * DK], FP32)
        nc.sync.dma_start(wq_sb[:], wq_d[:].rearrange("(c p) n -> p c n", p=128))
        wk_sb = con.tile([128, 4, HPD * DK], FP32)
        nc.sync.dma_start(wk_sb[:], wk_d[:].rearrange("(c p) n -> p c n", p=128))
        wv_sb = con.tile([128, 4, HPD * DK], FP32)
        nc.sync.dma_start(wv_sb[:], wv_d[:].rearrange("(c p) n -> p c n", p=128))
        wc_sb = con.tile([128, K * DK], FP32)      # wconv_t replicated on both halves
        nc.sync.dma_start(wc_sb[:], wc_d[:])
        wo_sb = con.tile([DK, HPD, D], FP32)
        nc.sync.dma_start(wo_sb[:], wo_d[:].rearrange("(h c) n -> c h n", h=HPD))
        bk_sb = con.tile([128, 2], FP32)
        nc.sync.dma_start(bk_sb[:], bk_d[:])
        bv_sb = con.tile([128, 2], FP32)
        nc.sync.dma_start(bv_sb[:], bv_d[:])
        cb_sb = con.tile([128, 1], FP32)
        nc.sync.dma_start(cb_sb[:], cb_d[:])
        bq_sb = con.tile([128, HPD * DK], FP32)
        nc.sync.dma_start(bq_sb[:], bq_d[:].partition_broadcast(128))

        # persistent pair-stacked tensors: rows 0:64 = head 2p, 64:128 = head 2p+1
        qnT = [persist.tile([128, S], FP32, tag=f"qnT{p}", name=f"qnT{p}")
               for p in range(2)]
        kT = [persist.tile([128, S], FP32, tag=f"kT{p}", name=f"kT{p}")
              for p in range(2)]
        IDXR = [persist.tile([128, NCHUNK, K], U16, tag=f"idxr{h}", name=f"idxr{h}")
                for h in range(HPD)]
        idxu = [persist.tile([128, K, NCHUNK], U16, tag=f"idxu{h}", name=f"idxu{h}")
                for h in range(HPD)]

        with tc.tile_pool(name="proj", bufs=2) as proj, \
             tc.tile_pool(name="projv", bufs=1) as projv, \
             tc.tile_pool(name="psA", bufs=2, space=bass.MemorySpace.PSUM) as psA:
            xT_sb = projv.tile([128, 4, S], FP32, tag="xT")
            nc.sync.dma_start(xT_sb[:], xT_d[:].rearrange("(c p) n -> p c n", p=128))
            vT = [projv.tile([128, S], FP32, tag=f"vT{p}", name=f"vT{p}")
                  for p in range(2)]

            # ---- q natural + bias + normalize + pair-transpose to qnT ----
            for cch in range(NCHUNK):
                qp = psA.tile([128, HPD * DK], FP32, tag="qproj")
                for kc in range(4):
                    nc.tensor.matmul(
                        qp[:], xT_sb[:, kc, cch * 128:(cch + 1) * 128],
                        wq_sb[:, kc, :], start=(kc == 0), stop=(kc == 3),
                    )
                qsb = proj.tile([128, HPD * DK], FP32, tag="qsb")
                nc.vector.tensor_add(qsb[:], qp[:], bq_sb[:])
                sq = proj.tile([128, HPD * DK], FP32, tag="sq")
                nc.vector.tensor_mul(sq[:], qsb[:], qsb[:])
                ssq = proj.tile([128, HPD], FP32, tag="ssq")
                nc.vector.reduce_sum(ssq[:], sq[:].rearrange("p (h c) -> p h c", h=HPD),
                                     axis=mybir.AxisListType.X)
                nrm = proj.tile([128, HPD], FP32, tag="nrm")
                nc.scalar.sqrt(nrm[:], ssq[:])
                nc.vector.tensor_scalar_max(nrm[:], nrm[:], 1e-12)
                rinv = proj.tile([128, HPD], FP32, tag="rinv")
                nc.vector.reciprocal(rinv[:], nrm[:])
                for h in range(HPD):
                    nc.vector.tensor_scalar_mul(
                        qsb[:, h * DK:(h + 1) * DK], qsb[:, h * DK:(h + 1) * DK],
                        rinv[:, h:h + 1])
                for p in range(2):
                    tp = psA.tile([128, 128], FP32, tag="qtp")
                    nc.tensor.transpose(tp[:], qsb[:, p * 128:(p + 1) * 128], ident[:])
                    if p == 0:
                        nc.scalar.copy(qnT[p][:, cch * 128:(cch + 1) * 128], tp[:])
                    else:
                        nc.vector.tensor_copy(qnT[p][:, cch * 128:(cch + 1) * 128], tp[:])

            # ---- kT / vT pair-stacked + bias ----
            for p in range(2):
                for s4 in range(4):
                    for (dst, w_sb, b_sb) in ((kT, wk_sb, bk_sb), (vT, wv_sb, bv_sb)):
                        kp = psA.tile([128, 512], FP32, tag="kproj")
                        for kc in range(4):
                            nc.tensor.matmul(
                                kp[:],
                                w_sb[:, kc, p * 128:(p + 1) * 128],
                                xT_sb[:, kc, s4 * 512:(s4 + 1) * 512],
                                start=(kc == 0), stop=(kc == 3),
                            )
                        nc.vector.tensor_scalar_add(
                            dst[p][:, s4 * 512:(s4 + 1) * 512], kp[:],
                            b_sb[:, p:p + 1])

            # ---- U tables: rows (s, kk) of u_kk^T -> DRAM [S*K, DK]; one DMA/head
            for h in range(HPD):
                p, q = h // 2, h % 2
                usb = projv.tile([128, NCHUNK, K * DK], FP32, tag="usb", bufs=1,
                                 name="usb")
                for cch in range(NCHUNK):
                    upA = psA.tile([128, 288], FP32, tag="uprojA", bufs=1)
                    upB = psA.tile([128, 288], FP32, tag="uprojB", bufs=1)
                    lhs = vT[p][q * DK:(q + 1) * DK, cch * 128:(cch + 1) * 128]
                    nc.tensor.matmul(upA[:], lhs, wc_sb[q * DK:(q + 1) * DK, 0:288],
                                     start=True, stop=True)
                    nc.tensor.matmul(upB[:], lhs, wc_sb[q * DK:(q + 1) * DK, 288:576],
                                     start=True, stop=True)
                    nc.scalar.copy(usb[:, cch, 0:288], upA[:])
                    nc.scalar.copy(usb[:, cch, 288:576], upB[:])
                nc.sync.dma_start(
                    utab_d[h][:].rearrange("(cc p n) c -> p cc (n c)", p=128, n=K),
                    usb[:])

        # ---- per-head similarity + top-9 + gather (overlapped) ----
        late = ctx.enter_context(tc.tile_pool(name="late", bufs=1))
        acc = [late.tile([128, NCHUNK, 128], FP32, tag=f"acc{p}", name=f"acc{p}")
               for p in range(2)]
        with tc.tile_pool(name="simpool", bufs=2, space=bass.MemorySpace.PSUM) as psS, \
             tc.tile_pool(name="topk", bufs=2) as tkp:
            for h in range(HPD):
                p, q = h // 2, h % 2
                scr9 = persist.tile([128, NCHUNK, 8], U16, tag="scr9", name="scr9")
                for cch in range(NCHUNK):
                    sim = psS.tile([128, S], FP32, tag="sim")
                    for jc in range(4):
                        nc.tensor.matmul(
                            sim[:, jc * 512:(jc + 1) * 512],
                            kT[p][q * DK:(q + 1) * DK, cch * 128:(cch + 1) * 128],
                            qnT[p][q * DK:(q + 1) * DK, jc * 512:(jc + 1) * 512],
                            start=True, stop=True,
                        )
                    cands = tkp.tile([128, NSEG * 8], FP32, tag="cands")
                    for sg in range(NSEG):
                        nc.vector.max(cands[:, sg * 8:(sg + 1) * 8],
                                      sim[:, sg * SEG:(sg + 1) * SEG])
                    g8 = tkp.tile([128, 8], FP32, tag="g8")
                    nc.vector.max(g8[:], cands[:])
                    c2 = tkp.tile([128, NSEG * 8], FP32, tag="c2")
                    nc.vector.match_replace(c2[:], g8[:], cands[:], NEG)
                    h8 = tkp.tile([128, 8], FP32, tag="h8")
                    nc.vector.max(h8[:], c2[:])
                    nc.vector.max_index(IDXR[h][:, cch, 0:8], g8[:], sim[:])
                    nc.vector.max_index(scr9[:, cch, :], h8[:], sim[:])
                nc.vector.tensor_copy(IDXR[h][:, :, 8], scr9[:, :, 0])
                idxf = tkp.tile([128, NCHUNK, K], FP32, tag="idxf")
                nc.vector.tensor_copy(idxf[:], IDXR[h][:])
                nc.vector.tensor_scalar(idxf[:], idxf[:], float(K), None,
                                        op0=mybir.AluOpType.mult)
                nc.vector.tensor_add(idxf[:], idxf[:], kkpat[:])
                nc.vector.tensor_copy(idxu[h][:].rearrange("p k g -> p g k"), idxf[:])
                nc.sync.dma_start(idxd_d[h][:], idxu[h][:].rearrange("p k g -> p (k g)"))
                p, q = h // 2, h % 2
                NIDX = S * K
                idx16 = late.tile([128, NIDX // 16], U16, tag="idx16", bufs=1,
                                  name="idx16")
                staged = late.tile([16, 8, K * NCHUNK], U16, tag="staged", bufs=1,
                                   name="staged")
                nc.sync.dma_start(
                    staged[:], idxd_d[h][:].rearrange("(w pp) f -> pp w f", w=8))
                nc.vector.tensor_copy(
                    idx16[0:16, :].rearrange("pp (k g w) -> pp k g w", k=K, w=8),
                    staged[:].rearrange("pp w (k g) -> pp k g w", k=K))
                nc.sync.dma_start(idx16[16:32, :], idx16[0:16, :])
                nc.sync.dma_start(idx16[32:64, :], idx16[0:32, :])
                nc.sync.dma_start(idx16[64:128, :], idx16[0:64, :])
                gball = late.tile([128, K, NCHUNK, DK], FP32, tag="gball", bufs=1,
                                  name="gball")
                gflat = gball[:].rearrange("p k g c -> p (k g) c")
                NG = 18
                step = NIDX // NG
                for j in range(NG):
                    nc.gpsimd.dma_gather(
                        out_ap=gflat[:, j * (step // 128):(j + 1) * (step // 128), :],
                        in_ap=utab_d[h][:],
                        idxs_ap=idx16[:, j * (step // 16):(j + 1) * (step // 16)].bitcast(I16),
                        num_idxs=step,
                        num_idxs_reg=step,
                        elem_size=DK,
                    )
                gb = [gball[:, kk, :, :] for kk in range(K)]
                nc.vector.tensor_add(gb[0], gb[0], gb[1])
                nc.gpsimd.tensor_add(gb[2], gb[2], gb[3])
                nc.vector.tensor_add(gb[4], gb[4], gb[5])
                nc.gpsimd.tensor_add(gb[6], gb[6], gb[7])
                nc.vector.tensor_add(gb[0], gb[0], gb[4])
                nc.gpsimd.tensor_add(gb[2], gb[2], gb[6])
                nc.vector.tensor_add(gb[0], gb[0], gb[8])
                nc.vector.tensor_add(acc[p][:, :, q * DK:(q + 1) * DK],
                                     gb[0], gb[2])

        # ---- transposes, scramble, Wo ----
        with tc.tile_pool(name="psB", bufs=2, space=bass.MemorySpace.PSUM) as psB:
            outT = [late.tile([128, S], FP32, tag=f"outT{p}", name=f"outT{p}")
                    for p in range(2)]
            for p in range(2):
                for cch in range(NCHUNK):
                    tp = psB.tile([128, 128], FP32, tag="otp")
                    nc.tensor.transpose(tp[:], acc[p][:, cch, :], ident[:])
                    if p == 0:
                        nc.scalar.add(outT[p][:, cch * 128:(cch + 1) * 128], tp[:],
                                      cb_sb[:, 0:1])
                    else:
                        nc.vector.tensor_scalar_add(
                            outT[p][:, cch * 128:(cch + 1) * 128], tp[:], cb_sb[:, 0:1])
            # scramble: AT_h[c2, 64a+r] = outT_pair[r(+64q), 32*c2+a]
            AT = [late.tile([DK, S], FP32, tag=f"AT{h}", name=f"AT{h}")
                  for h in range(HPD)]
            for h in range(HPD):
                p, q = h // 2, h % 2
                for a in range(32):
                    tp = psB.tile([DK, DK], FP32, tag="atp")
                    nc.tensor.transpose(tp[:], outT[p][q * DK:(q + 1) * DK, a:S:32],
                                        id2[q * DK:(q + 1) * DK, :])
                    if h % 2 == 0:
                        nc.scalar.copy(AT[h][:, a * DK:(a + 1) * DK], tp[:])
                    else:
                        nc.vector.tensor_copy(AT[h][:, a * DK:(a + 1) * DK], tp[:])
            # Wo partial projection (accumulate over 4 heads, K=64 each)
            for cch in range(NCHUNK):
                yp = psB.tile([128, D], FP32, tag="yp")
                for h in range(HPD):
                    nc.tensor.matmul(yp[:], AT[h][:, cch * 128:(cch + 1) * 128],
                                     wo_sb[:, h, :], start=(h == 0), stop=(h == HPD - 1))
                yb = late.tile([128, D], FP32, tag="ybounce", bufs=2, name="yb")
                nc.scalar.copy(yb[:], yp[:])
                nc.sync.dma_start(y_d[cch * 128:(cch + 1) * 128, :], yb[:])
            if debug:
                for h in range(HPD):
                    nc.sync.dma_start(idx_dbg[h][:], IDXR[h][:])
                for p in range(2):
                    nc.sync.dma_start(acc_dbg[p][:], acc[p][:])
                    nc.sync.dma_start(qnT_dbg[p][:], qnT[p][:])
                    nc.sync.dma_start(kT_dbg[p][:], kT[p][:])

    nc.compile()
    if split_waits:
        _split_multi_waits(nc)
    return nc


_CACHED = {}


def _get_program():
    if "nc" not in _CACHED:
        _CACHED["nc"] = build_program()
    return _CACHED["nc"]


def make_in_maps(x, Wq, bq, Wk, bk, Wv, bv, Wo, cw, cb):
    wconv_t = np.concatenate([cw[:, :, kk].T for kk in range(K)], axis=1)  # [64, 576]
    wconv_t2 = np.concatenate([wconv_t, wconv_t], axis=0)                  # [128, 576]
    cb2 = np.concatenate([cb, cb])[:, None]                                # [128, 1]
    in_maps = []
    for d in range(8):
        b, g = d // 2, d % 2
        rows = slice(g * HPD * DK, (g + 1) * HPD * DK)
        bk_l, bv_l = bk[rows], bv[rows]
        bk_pair = np.stack([bk_l[0:128], bk_l[128:256]], axis=1)
        bv_pair = np.stack([bv_l[0:128], bv_l[128:256]], axis=1)
        in_maps.append({
            "xT": np.ascontiguousarray(x[b].T),
            "wq_t": np.ascontiguousarray(Wq[rows].T),
            "wk_t": np.ascontiguousarray(Wk[rows].T),
            "wv_t": np.ascontiguousarray(Wv[rows].T),
            "bq": np.ascontiguousarray(bq[rows][None, :]),
            "bk_pair": np.ascontiguousarray(bk_pair),
            "bv_pair": np.ascontiguousarray(bv_pair),
            "wconv_t2": np.ascontiguousarray(wconv_t2),
            "conv_b2": np.ascontiguousarray(cb2),
            "wo_t": np.ascontiguousarray(Wo[:, rows].T),
        })
    return in_maps


def kernel(**inputs):
    from concourse.bass_utils import run_bass_kernel_spmd

    x = np.asarray(inputs["x"], np.float32)
    Wq = np.asarray(inputs["Wq_w"], np.float32)
    bq = np.asarray(inputs["Wq_b"], np.float32)
    Wk = np.asarray(inputs["Wk_w"], np.float32)
    bk = np.asarray(inputs["Wk_b"], np.float32)
    Wv = np.asarray(inputs["Wv_w"], np.float32)
    bv = np.asarray(inputs["Wv_b"], np.float32)
    Wo = np.asarray(inputs["Wo_w"], np.float32)
    bo = np.asarray(inputs["Wo_b"], np.float32)
    cw = np.asarray(inputs["conv_w"], np.float32)
    cb = np.asarray(inputs["conv_b"], np.float32)

    nc = _get_program()
    in_maps = make_in_maps(x, Wq, bq, Wk, bk, Wv, bv, Wo, cw, cb)
    res = run_bass_kernel_spmd(nc, in_maps, core_ids=list(range(8)))
    y = np.zeros((B, S, D), np.float32)
    for b in range(B):
        y[b] = res.results[2 * b]["y"] + res.results[2 * b + 1]["y"] + bo[None, :]
    return y


if __name__ == "__main__":
    nc = build_program()
    print("program built ok")


# revision 28
# speedup vs baseline: 2.2999x; 1.3021x over previous
"""Trainium2 Bass kernel for nn_MultiHeadConvNNAttention.

Sharding: 8 cores; core d handles batch b = d//2 and head-group g = d%2
(4 heads of H=8 each). Per core: q/k/v projections (fp32 on PE), per-head
SxS similarity (k^T @ qn, fp32), exact top-9 per row via segmented DVE
max8/max_index, neighbor-conv as 9 accumulating indirect-DMA gathers from a
precomputed u-table in DRAM, the torch-view output scramble via PE
transposes, and a partial Wo projection. Host sums the two partial y's per
batch and adds Wo_b.

Self-contained: hardcodes all shapes; does not read /root/problem files.
"""
import sys
import numpy as np

sys.path.insert(0, "/opt/trn_rl_repo")

import bass_rust
import concourse.bass as bass
import concourse.bacc as bacc_mod
import concourse.mybir as mybir
import concourse.tile as tile
from concourse.masks import make_identity
from contextlib import ExitStack

B, S, D, H, K = 4, 2048, 512, 8, 9
DK = D // H          # 64
HPD = H // 2         # heads per device = 4
NCHUNK = S // 128    # 16 i-chunks
NSEG = 8             # row segments for seg-max8
SEG = S // NSEG      # 256
FP32 = mybir.dt.float32
U16 = mybir.dt.uint16
I16 = mybir.dt.int16
U32 = mybir.dt.uint32
I32 = mybir.dt.int32
NEG = -3.0e38


def _split_multi_waits(nc):
    """This walrus build supports at most one sem-wait per instruction;
    split extras onto preceding same-engine drain carriers."""
    n = [0]

    def fix_block(blk):
        insts = blk.instructions
        out = []
        changed = False
        for i in insts:
            si = i.sync_info
            ow = list(si.on_wait) if si is not None and si.on_wait is not None else []
            if len(ow) > 1:
                changed = True
                for w in ow[:-1]:
                    n[0] += 1
                    c = mybir.InstDrain(name=f"wsplit_{n[0]}", ins=[], outs=[])
                    c.engine = i.engine
                    c.sync_info = bass_rust.SyncInfo(on_wait=[w], on_update=[])
                    out.append(c)
                i.sync_info = bass_rust.SyncInfo(
                    on_wait=[ow[-1]], on_update=list(si.on_update or []))
            out.append(i)
        if changed:
            blk.instructions = out

    for f in nc.m.functions:
        for blk in f.blocks:
            fix_block(blk)
    return n[0]


def build_program(split_waits=True, debug=False):
    nc = bacc_mod.Bacc()

    # ---- DRAM I/O ----
    xT_d = nc.dram_tensor("xT", [D, S], FP32, kind="ExternalInput")
    wq_d = nc.dram_tensor("wq_t", [D, HPD * DK], FP32, kind="ExternalInput")
    wk_d = nc.dram_tensor("wk_t", [D, HPD * DK], FP32, kind="ExternalInput")
    wv_d = nc.dram_tensor("wv_t", [D, HPD * DK], FP32, kind="ExternalInput")
    bq_d = nc.dram_tensor("bq", [1, HPD * DK], FP32, kind="ExternalInput")
    bk_d = nc.dram_tensor("bk_pair", [128, 2], FP32, kind="ExternalInput")
    bv_d = nc.dram_tensor("bv_pair", [128, 2], FP32, kind="ExternalInput")
    wc_d = nc.dram_tensor("wconv_t2", [128, K * DK], FP32, kind="ExternalInput")
    cb_d = nc.dram_tensor("conv_b2", [128, 1], FP32, kind="ExternalInput")
    wo_d = nc.dram_tensor("wo_t", [HPD * DK, D], FP32, kind="ExternalInput")
    y_d = nc.dram_tensor("y", [S, D], FP32, kind="ExternalOutput")
    utab_d = [
        nc.dram_tensor(f"utab{h}", [S * K, DK], FP32,
                       kind=("ExternalOutput" if debug else "Internal"))
        for h in range(HPD)
    ]
    idxd_d = [nc.dram_tensor(f"idxd{h}", [128, K * NCHUNK], U16, kind="Internal")
              for h in range(HPD)]
    if debug:
        idx_dbg = [nc.dram_tensor(f"idxdbg{h}", [128, NCHUNK, K], U16,
                                  kind="ExternalOutput") for h in range(HPD)]
        acc_dbg = [nc.dram_tensor(f"accdbg{p}", [128, NCHUNK, 128], FP32,
                                  kind="ExternalOutput") for p in range(2)]
        qnT_dbg = [nc.dram_tensor(f"qnTdbg{p}", [128, S], FP32,
                                  kind="ExternalOutput") for p in range(2)]
        kT_dbg = [nc.dram_tensor(f"kTdbg{p}", [128, S], FP32,
                                 kind="ExternalOutput") for p in range(2)]

    with tile.TileContext(nc) as tc, ExitStack() as ctx:
        con = ctx.enter_context(tc.tile_pool(name="consts", bufs=1))
        persist = ctx.enter_context(tc.tile_pool(name="persist", bufs=1))

        # ---- constants ----
        ident = con.tile([128, 128], FP32)
        make_identity(nc, ident[:])
        id2 = con.tile([128, DK], FP32)   # id2[p, j] = (p % 64 == j)
        make_identity(nc, id2[0:DK, :])
        make_identity(nc, id2[DK:128, :])
        kkpat_i = con.tile([128, NCHUNK, K], I32)
        nc.gpsimd.iota(kkpat_i[:], pattern=[[0, NCHUNK], [1, K]], base=0,
                       channel_multiplier=0)
        kkpat = con.tile([128, NCHUNK, K], FP32)
        nc.vector.tensor_copy(kkpat[:], kkpat_i[:])

        # weights / biases to SBUF
        wq_sb = con.tile([128, 4, HPD * DK], FP32)
        nc.sync.dma_start(wq_sb[:], wq_d[:].rearrange("(c p) n -> p c n", p=128))
        wk_sb = con.tile([128, 4, HPD * DK], FP32)
        nc.sync.dma_start(wk_sb[:], wk_d[:].rearrange("(c p) n -> p c n", p=128))
        wv_sb = con.tile([128, 4, HPD * DK], FP32)
        nc.sync.dma_start(wv_sb[:], wv_d[:].rearrange("(c p) n -> p c n", p=128))
        wc_sb = con.tile([128, K * DK], FP32)      # wconv_t replicated on both halves
        nc.sync.dma_start(wc_sb[:], wc_d[:])
        wo_sb = con.tile([DK, HPD, D], FP32)
        nc.sync.dma_start(wo_sb[:], wo_d[:].rearrange("(h c) n -> c h n", h=HPD))
        bk_sb = con.tile([128, 2], FP32)
        nc.sync.dma_start(bk_sb[:], bk_d[:])
        bv_sb = con.tile([128, 2], FP32)
        nc.sync.dma_start(bv_sb[:], bv_d[:])
        cb_sb = con.tile([128, 1], FP32)
        nc.sync.dma_start(cb_sb[:], cb_d[:])
        bq_sb = con.tile([128, HPD * DK], FP32)
        nc.sync.dma_start(bq_sb[:], bq_d[:].partition_broadcast(128))

        # persistent pair-stacked tensors: rows 0:64 = head 2p, 64:128 = head 2p+1
        qnT = [persist.tile([128, S], FP32, tag=f"qnT{p}", name=f"qnT{p}")
               for p in range(2)]
        kT = [persist.tile([128, S], FP32, tag=f"kT{p}", name=f"kT{p}")
              for p in range(2)]
        IDXR = [persist.tile([128, NCHUNK, K], U16, tag=f"idxr{h}", name=f"idxr{h}")
                for h in range(HPD)]
        idxu = [persist.tile([128, K, NCHUNK], U16, tag=f"idxu{h}", name=f"idxu{h}")
                for h in range(HPD)]

        with tc.tile_pool(name="proj", bufs=2) as proj, \
             tc.tile_pool(name="projv", bufs=1) as projv, \
             tc.tile_pool(name="psA", bufs=2, space=bass.MemorySpace.PSUM) as psA:
            xT_sb = projv.tile([128, 4, S], FP32, tag="xT")
            nc.sync.dma_start(xT_sb[:], xT_d[:].rearrange("(c p) n -> p c n", p=128))
            vT = [projv.tile([128, S], FP32, tag=f"vT{p}", name=f"vT{p}")
                  for p in range(2)]

            # ---- q natural + bias + normalize + pair-transpose to qnT ----
            for cch in range(NCHUNK):
                qp = psA.tile([128, HPD * DK], FP32, tag="qproj")
                for kc in range(4):
                    nc.tensor.matmul(
                        qp[:], xT_sb[:, kc, cch * 128:(cch + 1) * 128],
                        wq_sb[:, kc, :], start=(kc == 0), stop=(kc == 3),
                    )
                qsb = proj.tile([128, HPD * DK], FP32, tag="qsb")
                nc.vector.tensor_add(qsb[:], qp[:], bq_sb[:])
                sq = proj.tile([128, HPD * DK], FP32, tag="sq")
                nc.vector.tensor_mul(sq[:], qsb[:], qsb[:])
                ssq = proj.tile([128, HPD], FP32, tag="ssq")
                nc.vector.reduce_sum(ssq[:], sq[:].rearrange("p (h c) -> p h c", h=HPD),
                                     axis=mybir.AxisListType.X)
                nrm = proj.tile([128, HPD], FP32, tag="nrm")
                nc.scalar.sqrt(nrm[:], ssq[:])
                nc.vector.tensor_scalar_max(nrm[:], nrm[:], 1e-12)
                rinv = proj.tile([128, HPD], FP32, tag="rinv")
                nc.vector.reciprocal(rinv[:], nrm[:])
                for h in range(HPD):
                    nc.vector.tensor_scalar_mul(
                        qsb[:, h * DK:(h + 1) * DK], qsb[:, h * DK:(h + 1) * DK],
                        rinv[:, h:h + 1])
                for p in range(2):
                    tp = psA.tile([128, 128], FP32, tag="qtp")
                    nc.tensor.transpose(tp[:], qsb[:, p * 128:(p + 1) * 128], ident[:])
                    if p == 0:
                        nc.scalar.copy(qnT[p][:, cch * 128:(cch + 1) * 128], tp[:])
                    else:
                        nc.vector.tensor_copy(qnT[p][:, cch * 128:(cch + 1) * 128], tp[:])

            # ---- kT / vT pair-stacked + bias ----
            for p in range(2):
                for s4 in range(4):
                    for (dst, w_sb, b_sb) in ((kT, wk_sb, bk_sb), (vT, wv_sb, bv_sb)):
                        kp = psA.tile([128, 512], FP32, tag="kproj")
                        for kc in range(4):
                            nc.tensor.matmul(
                                kp[:],
                                w_sb[:, kc, p * 128:(p + 1) * 128],
                                xT_sb[:, kc, s4 * 512:(s4 + 1) * 512],
                                start=(kc == 0), stop=(kc == 3),
                            )
                        nc.vector.tensor_scalar_add(
                            dst[p][:, s4 * 512:(s4 + 1) * 512], kp[:],
                            b_sb[:, p:p + 1])

            # ---- U tables: rows (s, kk) of u_kk^T -> DRAM [S*K, DK]; one DMA/head
            for h in range(HPD):
                p, q = h // 2, h % 2
                usb = projv.tile([128, NCHUNK, K * DK], FP32, tag="usb", bufs=1,
                                 name="usb")
                for cch in range(NCHUNK):
                    upA = psA.tile([128, 288], FP32, tag="uprojA", bufs=1)
                    upB = psA.tile([128, 288], FP32, tag="uprojB", bufs=1)
                    lhs = vT[p][q * DK:(q + 1) * DK, cch * 128:(cch + 1) * 128]
                    nc.tensor.matmul(upA[:], lhs, wc_sb[q * DK:(q + 1) * DK, 0:288],
                                     start=True, stop=True)
                    nc.tensor.matmul(upB[:], lhs, wc_sb[q * DK:(q + 1) * DK, 288:576],
                                     start=True, stop=True)
                    nc.scalar.copy(usb[:, cch, 0:288], upA[:])
                    nc.scalar.copy(usb[:, cch, 288:576], upB[:])
                nc.sync.dma_start(
                    utab_d[h][:].rearrange("(cc p n) c -> p cc (n c)", p=128, n=K),
                    usb[:])

        # ---- per-head similarity + top-9 + gather (overlapped) ----
        late = ctx.enter_context(tc.tile_pool(name="late", bufs=1))
        acc = [late.tile([128, NCHUNK, 128], FP32, tag=f"acc{p}", name=f"acc{p}")
               for p in range(2)]
        with tc.tile_pool(name="simpool", bufs=2, space=bass.MemorySpace.PSUM) as psS, \
             tc.tile_pool(name="topk", bufs=2) as tkp:
            for h in range(HPD):
                p, q = h // 2, h % 2
                scr9 = persist.tile([128, NCHUNK, 8], U16, tag="scr9", name="scr9")
                for cch in range(NCHUNK):
                    sim = psS.tile([128, S], FP32, tag="sim")
                    for jc in range(4):
                        nc.tensor.matmul(
                            sim[:, jc * 512:(jc + 1) * 512],
                            kT[p][q * DK:(q + 1) * DK, cch * 128:(cch + 1) * 128],
                            qnT[p][q * DK:(q + 1) * DK, jc * 512:(jc + 1) * 512],
                            start=True, stop=True,
                        )
                    cands = tkp.tile([128, NSEG * 8], FP32, tag="cands")
                    for sg in range(NSEG):
                        nc.vector.max(cands[:, sg * 8:(sg + 1) * 8],
                                      sim[:, sg * SEG:(sg + 1) * SEG])
                    g8 = tkp.tile([128, 8], FP32, tag="g8")
                    nc.vector.max(g8[:], cands[:])
                    c2 = tkp.tile([128, NSEG * 8], FP32, tag="c2")
                    nc.vector.match_replace(c2[:], g8[:], cands[:], NEG)
                    h8 = tkp.tile([128, 8], FP32, tag="h8")
                    nc.vector.max(h8[:], c2[:])
                    nc.vector.max_index(IDXR[h][:, cch, 0:8], g8[:], sim[:])
                    nc.vector.max_index(scr9[:, cch, :], h8[:], sim[:])
                nc.vector.tensor_copy(IDXR[h][:, :, 8], scr9[:, :, 0])
                idxf = tkp.tile([128, NCHUNK, K], FP32, tag="idxf")
                nc.vector.tensor_copy(idxf[:], IDXR[h][:])
                nc.vector.tensor_scalar(idxf[:], idxf[:], float(K), None,
                                        op0=mybir.AluOpType.mult)
                nc.vector.tensor_add(idxf[:], idxf[:], kkpat[:])
                nc.vector.tensor_copy(idxu[h][:].rearrange("p k g -> p g k"), idxf[:])
                nc.sync.dma_start(idxd_d[h][:], idxu[h][:].rearrange("p k g -> p (k g)"))
                p, q = h // 2, h % 2
                NIDX = S * K
                idx16 = late.tile([128, NIDX // 16], U16, tag="idx16", bufs=1,
                                  name="idx16")
                staged = late.tile([16, 8, K * NCHUNK], U16, tag="staged", bufs=1,
                                   name="staged")
                nc.sync.dma_start(
                    staged[:], idxd_d[h][:].rearrange("(w pp) f -> pp w f", w=8))
                nc.vector.tensor_copy(
                    idx16[0:16, :].rearrange("pp (k g w) -> pp k g w", k=K, w=8),
                    staged[:].rearrange("pp w (k g) -> pp k g w", k=K))
                nc.sync.dma_start(idx16[16:32, :], idx16[0:16, :])
                nc.sync.dma_start(idx16[32:64, :], idx16[0:32, :])
                nc.sync.dma_start(idx16[64:128, :], idx16[0:64, :])
                gball = late.tile([128, K, NCHUNK, DK], FP32, tag="gball", bufs=1,
                                  name="gball")
                gflat = gball[:].rearrange("p k g c -> p (k g) c")
                NG = 18
                step = NIDX // NG
                for j in range(NG):
                    nc.gpsimd.dma_gather(
                        out_ap=gflat[:, j * (step // 128):(j + 1) * (step // 128), :],
                        in_ap=utab_d[h][:],
                        idxs_ap=idx16[:, j * (step // 16):(j + 1) * (step // 16)].bitcast(I16),
                        num_idxs=step,
                        num_idxs_reg=step,
                        elem_size=DK,
                    )
                gb = [gball[:, kk, :, :] for kk in range(K)]
                nc.gpsimd.tensor_add(gb[0], gb[0], gb[1])
                nc.gpsimd.tensor_add(gb[2], gb[2], gb[3])
                nc.gpsimd.tensor_add(gb[4], gb[4], gb[5])
                nc.gpsimd.tensor_add(gb[6], gb[6], gb[7])
                nc.gpsimd.tensor_add(gb[0], gb[0], gb[4])
                nc.gpsimd.tensor_add(gb[2], gb[2], gb[6])
                nc.gpsimd.tensor_add(gb[0], gb[0], gb[8])
                nc.gpsimd.tensor_add(acc[p][:, :, q * DK:(q + 1) * DK],
                                     gb[0], gb[2])

        # ---- transposes, scramble, Wo ----
        with tc.tile_pool(name="psB", bufs=2, space=bass.MemorySpace.PSUM) as psB:
            outT = [late.tile([128, S], FP32, tag=f"outT{p}", name=f"outT{p}")
                    for p in range(2)]
            for p in range(2):
                for cch in range(NCHUNK):
                    tp = psB.tile([128, 128], FP32, tag="otp")
                    nc.tensor.transpose(tp[:], acc[p][:, cch, :], ident[:])
                    if p == 0:
                        nc.scalar.add(outT[p][:, cch * 128:(cch + 1) * 128], tp[:],
                                      cb_sb[:, 0:1])
                    else:
                        nc.vector.tensor_scalar_add(
                            outT[p][:, cch * 128:(cch + 1) * 128], tp[:], cb_sb[:, 0:1])
            # scramble: AT_h[c2, 64a+r] = outT_pair[r(+64q), 32*c2+a]
            AT = [late.tile([DK, S], FP32, tag=f"AT{h}", name=f"AT{h}")
                  for h in range(HPD)]
            for h in range(HPD):
                p, q = h // 2, h % 2
                for a in range(32):
                    tp = psB.tile([DK, DK], FP32, tag="atp")
                    nc.tensor.transpose(tp[:], outT[p][q * DK:(q + 1) * DK, a:S:32],
                                        id2[q * DK:(q + 1) * DK, :])
                    if h % 2 == 0:
                        nc.scalar.copy(AT[h][:, a * DK:(a + 1) * DK], tp[:])
                    else:
                        nc.vector.tensor_copy(AT[h][:, a * DK:(a + 1) * DK], tp[:])
            # Wo partial projection (accumulate over 4 heads, K=64 each)
            for cch in range(NCHUNK):
                yp = psB.tile([128, D], FP32, tag="yp")
                for h in range(HPD):
                    nc.tensor.matmul(yp[:], AT[h][:, cch * 128:(cch + 1) * 128],
                                     wo_sb[:, h, :], start=(h == 0), stop=(h == HPD - 1))
                yb = late.tile([128, D], FP32, tag="ybounce", bufs=2, name="yb")
                nc.scalar.copy(yb[:], yp[:])
                nc.sync.dma_start(y_d[cch * 128:(cch + 1) * 128, :], yb[:])
            if debug:
                for h in range(HPD):
                    nc.sync.dma_start(idx_dbg[h][:], IDXR[h][:])
                for p in range(2):
                    nc.sync.dma_start(acc_dbg[p][:], acc[p][:])
                    nc.sync.dma_start(qnT_dbg[p][:], qnT[p][:])
                    nc.sync.dma_start(kT_dbg[p][:], kT[p][:])

    nc.compile()
    if split_waits:
        _split_multi_waits(nc)
    return nc


_CACHED = {}


def _get_program():
    if "nc" not in _CACHED:
        _CACHED["nc"] = build_program()
    return _CACHED["nc"]


def make_in_maps(x, Wq, bq, Wk, bk, Wv, bv, Wo, cw, cb):
    wconv_t = np.concatenate([cw[:, :, kk].T for kk in range(K)], axis=1)  # [64, 576]
    wconv_t2 = np.concatenate([wconv_t, wconv_t], axis=0)                  # [128, 576]
    cb2 = np.concatenate([cb, cb])[:, None]                                # [128, 1]
    in_maps = []
    for d in range(8):
        b, g = d // 2, d % 2
        rows = slice(g * HPD * DK, (g + 1) * HPD * DK)
        bk_l, bv_l = bk[rows], bv[rows]
        bk_pair = np.stack([bk_l[0:128], bk_l[128:256]], axis=1)
        bv_pair = np.stack([bv_l[0:128], bv_l[128:256]], axis=1)
        in_maps.append({
            "xT": np.ascontiguousarray(x[b].T),
            "wq_t": np.ascontiguousarray(Wq[rows].T),
            "wk_t": np.ascontiguousarray(Wk[rows].T),
            "wv_t": np.ascontiguousarray(Wv[rows].T),
            "bq": np.ascontiguousarray(bq[rows][None, :]),
            "bk_pair": np.ascontiguousarray(bk_pair),
            "bv_pair": np.ascontiguousarray(bv_pair),
            "wconv_t2": np.ascontiguousarray(wconv_t2),
            "conv_b2": np.ascontiguousarray(cb2),
            "wo_t": np.ascontiguousarray(Wo[:, rows].T),
        })
    return in_maps


def kernel(**inputs):
    from concourse.bass_utils import run_bass_kernel_spmd

    x = np.asarray(inputs["x"], np.float32)
    Wq = np.asarray(inputs["Wq_w"], np.float32)
    bq = np.asarray(inputs["Wq_b"], np.float32)
    Wk = np.asarray(inputs["Wk_w"], np.float32)
    bk = np.asarray(inputs["Wk_b"], np.float32)
    Wv = np.asarray(inputs["Wv_w"], np.float32)
    bv = np.asarray(inputs["Wv_b"], np.float32)
    Wo = np.asarray(inputs["Wo_w"], np.float32)
    bo = np.asarray(inputs["Wo_b"], np.float32)
    cw = np.asarray(inputs["conv_w"], np.float32)
    cb = np.asarray(inputs["conv_b"], np.float32)

    nc = _get_program()
    in_maps = make_in_maps(x, Wq, bq, Wk, bk, Wv, bv, Wo, cw, cb)
    res = run_bass_kernel_spmd(nc, in_maps, core_ids=list(range(8)))
    y = np.zeros((B, S, D), np.float32)
    for b in range(B):
        y[b] = res.results[2 * b]["y"] + res.results[2 * b + 1]["y"] + bo[None, :]
    return y


if __name__ == "__main__":
    nc = build_program()
    print("program built ok")


# revision 29
# speedup vs baseline: 2.5422x; 1.1054x over previous
"""Trainium2 Bass kernel for nn_MultiHeadConvNNAttention.

Sharding: 8 cores; core d handles batch b = d//2 and head-group g = d%2
(4 heads of H=8 each). Per core: q/k/v projections (fp32 on PE), per-head
SxS similarity (k^T @ qn, fp32), exact top-9 per row via segmented DVE
max8/max_index, neighbor-conv as 9 accumulating indirect-DMA gathers from a
precomputed u-table in DRAM, the torch-view output scramble via PE
transposes, and a partial Wo projection. Host sums the two partial y's per
batch and adds Wo_b.

Self-contained: hardcodes all shapes; does not read /root/problem files.
"""
import sys
import numpy as np
import ml_dtypes

sys.path.insert(0, "/opt/trn_rl_repo")

import bass_rust
import concourse.bass as bass
import concourse.bacc as bacc_mod
import concourse.mybir as mybir
import concourse.tile as tile
from concourse.masks import make_identity
from contextlib import ExitStack

B, S, D, H, K = 4, 2048, 512, 8, 9
DK = D // H          # 64
HPD = H // 2         # heads per device = 4
NCHUNK = S // 128    # 16 i-chunks
NSEG = 8             # row segments for seg-max8
SEG = S // NSEG      # 256
FP32 = mybir.dt.float32
U16 = mybir.dt.uint16
I16 = mybir.dt.int16
U32 = mybir.dt.uint32
I32 = mybir.dt.int32
BF16 = mybir.dt.bfloat16
NEG = -3.0e38


def _split_multi_waits(nc):
    """This walrus build supports at most one sem-wait per instruction;
    split extras onto preceding same-engine drain carriers."""
    n = [0]

    def fix_block(blk):
        insts = blk.instructions
        out = []
        changed = False
        for i in insts:
            si = i.sync_info
            ow = list(si.on_wait) if si is not None and si.on_wait is not None else []
            if len(ow) > 1:
                changed = True
                for w in ow[:-1]:
                    n[0] += 1
                    c = mybir.InstDrain(name=f"wsplit_{n[0]}", ins=[], outs=[])
                    c.engine = i.engine
                    c.sync_info = bass_rust.SyncInfo(on_wait=[w], on_update=[])
                    out.append(c)
                i.sync_info = bass_rust.SyncInfo(
                    on_wait=[ow[-1]], on_update=list(si.on_update or []))
            out.append(i)
        if changed:
            blk.instructions = out

    for f in nc.m.functions:
        for blk in f.blocks:
            fix_block(blk)
    return n[0]


def build_program(split_waits=True, debug=False):
    nc = bacc_mod.Bacc()

    # ---- DRAM I/O ----
    xT_d = nc.dram_tensor("xT", [D, S], FP32, kind="ExternalInput")
    wq_d = nc.dram_tensor("wq_t", [D, HPD * DK], FP32, kind="ExternalInput")
    wk_d = nc.dram_tensor("wk_t", [D, HPD * DK], FP32, kind="ExternalInput")
    wv_d = nc.dram_tensor("wv_t", [D, HPD * DK], FP32, kind="ExternalInput")
    bq_d = nc.dram_tensor("bq", [1, HPD * DK], FP32, kind="ExternalInput")
    bk_d = nc.dram_tensor("bk_pair", [128, 2], FP32, kind="ExternalInput")
    bv_d = nc.dram_tensor("bv_pair", [128, 2], FP32, kind="ExternalInput")
    wc_d = nc.dram_tensor("wconv_t2", [128, K * DK], BF16, kind="ExternalInput")
    cb_d = nc.dram_tensor("conv_b2", [128, 1], FP32, kind="ExternalInput")
    wo_d = nc.dram_tensor("wo_t", [HPD * DK, D], BF16, kind="ExternalInput")
    y_d = nc.dram_tensor("y", [S, D], FP32, kind="ExternalOutput")
    utab_d = [
        nc.dram_tensor(f"utab{h}", [S * K, DK], FP32,
                       kind=("ExternalOutput" if debug else "Internal"))
        for h in range(HPD)
    ]
    idxd_d = [nc.dram_tensor(f"idxd{h}", [128, K * NCHUNK], U16, kind="Internal")
              for h in range(HPD)]
    if debug:
        idx_dbg = [nc.dram_tensor(f"idxdbg{h}", [128, NCHUNK, K], U16,
                                  kind="ExternalOutput") for h in range(HPD)]
        acc_dbg = [nc.dram_tensor(f"accdbg{p}", [128, NCHUNK, 128], FP32,
                                  kind="ExternalOutput") for p in range(2)]
        qnT_dbg = [nc.dram_tensor(f"qnTdbg{p}", [128, S], FP32,
                                  kind="ExternalOutput") for p in range(2)]
        kT_dbg = [nc.dram_tensor(f"kTdbg{p}", [128, S], FP32,
                                 kind="ExternalOutput") for p in range(2)]

    with tile.TileContext(nc) as tc, ExitStack() as ctx:
        con = ctx.enter_context(tc.tile_pool(name="consts", bufs=1))
        persist = ctx.enter_context(tc.tile_pool(name="persist", bufs=1))

        # ---- constants ----
        ident = con.tile([128, 128], FP32)
        make_identity(nc, ident[:])
        id2 = con.tile([128, DK], FP32)   # id2[p, j] = (p % 64 == j)
        make_identity(nc, id2[0:DK, :])
        make_identity(nc, id2[DK:128, :])
        kkpat_i = con.tile([128, NCHUNK, K], I32)
        nc.gpsimd.iota(kkpat_i[:], pattern=[[0, NCHUNK], [1, K]], base=0,
                       channel_multiplier=0)
        kkpat = con.tile([128, NCHUNK, K], FP32)
        nc.vector.tensor_copy(kkpat[:], kkpat_i[:])

        # weights / biases to SBUF
        wq_sb = con.tile([128, 4, HPD * DK], FP32)
        nc.sync.dma_start(wq_sb[:], wq_d[:].rearrange("(c p) n -> p c n", p=128))
        wk_sb = con.tile([128, 4, HPD * DK], FP32)
        nc.sync.dma_start(wk_sb[:], wk_d[:].rearrange("(c p) n -> p c n", p=128))
        wv_sb = con.tile([128, 4, HPD * DK], FP32)
        nc.sync.dma_start(wv_sb[:], wv_d[:].rearrange("(c p) n -> p c n", p=128))
        wc_sb = con.tile([128, K * DK], BF16)      # wconv_t replicated on both halves
        nc.sync.dma_start(wc_sb[:], wc_d[:])
        wo_sb = con.tile([DK, HPD, D], BF16)
        nc.sync.dma_start(wo_sb[:], wo_d[:].rearrange("(h c) n -> c h n", h=HPD))
        bk_sb = con.tile([128, 2], FP32)
        nc.sync.dma_start(bk_sb[:], bk_d[:])
        bv_sb = con.tile([128, 2], FP32)
        nc.sync.dma_start(bv_sb[:], bv_d[:])
        cb_sb = con.tile([128, 1], FP32)
        nc.sync.dma_start(cb_sb[:], cb_d[:])
        bq_sb = con.tile([128, HPD * DK], FP32)
        nc.sync.dma_start(bq_sb[:], bq_d[:].partition_broadcast(128))

        # persistent pair-stacked tensors: rows 0:64 = head 2p, 64:128 = head 2p+1
        qnT = [persist.tile([128, S], FP32, tag=f"qnT{p}", name=f"qnT{p}")
               for p in range(2)]
        kT = [persist.tile([128, S], FP32, tag=f"kT{p}", name=f"kT{p}")
              for p in range(2)]
        IDXR = [persist.tile([128, NCHUNK, K], U16, tag=f"idxr{h}", name=f"idxr{h}")
                for h in range(HPD)]
        idxu = [persist.tile([128, K, NCHUNK], U16, tag=f"idxu{h}", name=f"idxu{h}")
                for h in range(HPD)]

        with tc.tile_pool(name="proj", bufs=2) as proj, \
             tc.tile_pool(name="projv", bufs=1) as projv, \
             tc.tile_pool(name="psA", bufs=2, space=bass.MemorySpace.PSUM) as psA:
            xT_sb = projv.tile([128, 4, S], FP32, tag="xT")
            nc.sync.dma_start(xT_sb[:], xT_d[:].rearrange("(c p) n -> p c n", p=128))
            vT = [projv.tile([128, S], BF16, tag=f"vT{p}", name=f"vT{p}")
                  for p in range(2)]

            # ---- q natural + bias + normalize + pair-transpose to qnT ----
            for cch in range(NCHUNK):
                qp = psA.tile([128, HPD * DK], FP32, tag="qproj")
                for kc in range(4):
                    nc.tensor.matmul(
                        qp[:], xT_sb[:, kc, cch * 128:(cch + 1) * 128],
                        wq_sb[:, kc, :], start=(kc == 0), stop=(kc == 3),
                    )
                qsb = proj.tile([128, HPD * DK], FP32, tag="qsb")
                nc.vector.tensor_add(qsb[:], qp[:], bq_sb[:])
                sq = proj.tile([128, HPD * DK], FP32, tag="sq")
                nc.vector.tensor_mul(sq[:], qsb[:], qsb[:])
                ssq = proj.tile([128, HPD], FP32, tag="ssq")
                nc.vector.reduce_sum(ssq[:], sq[:].rearrange("p (h c) -> p h c", h=HPD),
                                     axis=mybir.AxisListType.X)
                nrm = proj.tile([128, HPD], FP32, tag="nrm")
                nc.scalar.sqrt(nrm[:], ssq[:])
                nc.vector.tensor_scalar_max(nrm[:], nrm[:], 1e-12)
                rinv = proj.tile([128, HPD], FP32, tag="rinv")
                nc.vector.reciprocal(rinv[:], nrm[:])
                for h in range(HPD):
                    nc.vector.tensor_scalar_mul(
                        qsb[:, h * DK:(h + 1) * DK], qsb[:, h * DK:(h + 1) * DK],
                        rinv[:, h:h + 1])
                for p in range(2):
                    tp = psA.tile([128, 128], FP32, tag="qtp")
                    nc.tensor.transpose(tp[:], qsb[:, p * 128:(p + 1) * 128], ident[:])
                    if p == 0:
                        nc.scalar.copy(qnT[p][:, cch * 128:(cch + 1) * 128], tp[:])
                    else:
                        nc.vector.tensor_copy(qnT[p][:, cch * 128:(cch + 1) * 128], tp[:])

            # ---- kT / vT pair-stacked + bias ----
            for p in range(2):
                for s4 in range(4):
                    for (dst, w_sb, b_sb) in ((kT, wk_sb, bk_sb), (vT, wv_sb, bv_sb)):
                        kp = psA.tile([128, 512], FP32, tag="kproj")
                        for kc in range(4):
                            nc.tensor.matmul(
                                kp[:],
                                w_sb[:, kc, p * 128:(p + 1) * 128],
                                xT_sb[:, kc, s4 * 512:(s4 + 1) * 512],
                                start=(kc == 0), stop=(kc == 3),
                            )
                        nc.vector.tensor_scalar_add(
                            dst[p][:, s4 * 512:(s4 + 1) * 512], kp[:],
                            b_sb[:, p:p + 1])

            # ---- U tables: rows (s, kk) of u_kk^T -> DRAM [S*K, DK]; one DMA/head
            for h in range(HPD):
                p, q = h // 2, h % 2
                usb = projv.tile([128, NCHUNK, K * DK], FP32, tag="usb", bufs=1,
                                 name="usb")
                for cch in range(NCHUNK):
                    upA = psA.tile([128, 288], FP32, tag="uprojA", bufs=1)
                    upB = psA.tile([128, 288], FP32, tag="uprojB", bufs=1)
                    lhs = vT[p][q * DK:(q + 1) * DK, cch * 128:(cch + 1) * 128]
                    nc.tensor.matmul(upA[:], lhs, wc_sb[q * DK:(q + 1) * DK, 0:288],
                                     start=True, stop=True)
                    nc.tensor.matmul(upB[:], lhs, wc_sb[q * DK:(q + 1) * DK, 288:576],
                                     start=True, stop=True)
                    nc.scalar.copy(usb[:, cch, 0:288], upA[:])
                    nc.scalar.copy(usb[:, cch, 288:576], upB[:])
                nc.sync.dma_start(
                    utab_d[h][:].rearrange("(cc p n) c -> p cc (n c)", p=128, n=K),
                    usb[:])

        # ---- per-head similarity + top-9 + gather (overlapped) ----
        late = ctx.enter_context(tc.tile_pool(name="late", bufs=1))
        acc = [late.tile([128, NCHUNK, 128], FP32, tag=f"acc{p}", name=f"acc{p}")
               for p in range(2)]
        with tc.tile_pool(name="simpool", bufs=2, space=bass.MemorySpace.PSUM) as psS, \
             tc.tile_pool(name="topk", bufs=2) as tkp:
            for h in range(HPD):
                p, q = h // 2, h % 2
                scr9 = persist.tile([128, NCHUNK, 8], U16, tag="scr9", name="scr9")
                for cch in range(NCHUNK):
                    sim = psS.tile([128, S], FP32, tag="sim")
                    for jc in range(4):
                        nc.tensor.matmul(
                            sim[:, jc * 512:(jc + 1) * 512],
                            kT[p][q * DK:(q + 1) * DK, cch * 128:(cch + 1) * 128],
                            qnT[p][q * DK:(q + 1) * DK, jc * 512:(jc + 1) * 512],
                            start=True, stop=True,
                        )
                    cands = tkp.tile([128, NSEG * 8], FP32, tag="cands")
                    for sg in range(NSEG):
                        nc.vector.max(cands[:, sg * 8:(sg + 1) * 8],
                                      sim[:, sg * SEG:(sg + 1) * SEG])
                    g8 = tkp.tile([128, 8], FP32, tag="g8")
                    nc.vector.max(g8[:], cands[:])
                    c2 = tkp.tile([128, NSEG * 8], FP32, tag="c2")
                    nc.vector.match_replace(c2[:], g8[:], cands[:], NEG)
                    h8 = tkp.tile([128, 8], FP32, tag="h8")
                    nc.vector.max(h8[:], c2[:])
                    nc.vector.max_index(IDXR[h][:, cch, 0:8], g8[:], sim[:])
                    nc.vector.max_index(scr9[:, cch, :], h8[:], sim[:])
                nc.vector.tensor_copy(IDXR[h][:, :, 8], scr9[:, :, 0])
                idxf = tkp.tile([128, NCHUNK, K], FP32, tag="idxf")
                nc.vector.tensor_copy(idxf[:], IDXR[h][:])
                nc.vector.tensor_scalar(idxf[:], idxf[:], float(K), None,
                                        op0=mybir.AluOpType.mult)
                nc.vector.tensor_add(idxf[:], idxf[:], kkpat[:])
                nc.vector.tensor_copy(idxu[h][:].rearrange("p k g -> p g k"), idxf[:])
                nc.sync.dma_start(idxd_d[h][:], idxu[h][:].rearrange("p k g -> p (k g)"))
                p, q = h // 2, h % 2
                NIDX = S * K
                idx16 = late.tile([128, NIDX // 16], U16, tag="idx16", bufs=1,
                                  name="idx16")
                staged = late.tile([16, 8, K * NCHUNK], U16, tag="staged", bufs=1,
                                   name="staged")
                nc.sync.dma_start(
                    staged[:], idxd_d[h][:].rearrange("(w pp) f -> pp w f", w=8))
                nc.vector.tensor_copy(
                    idx16[0:16, :].rearrange("pp (k g w) -> pp k g w", k=K, w=8),
                    staged[:].rearrange("pp w (k g) -> pp k g w", k=K))
                nc.sync.dma_start(idx16[16:32, :], idx16[0:16, :])
                nc.sync.dma_start(idx16[32:64, :], idx16[0:32, :])
                nc.sync.dma_start(idx16[64:128, :], idx16[0:64, :])
                gball = late.tile([128, K, NCHUNK, DK], FP32, tag="gball", bufs=1,
                                  name="gball")
                gflat = gball[:].rearrange("p k g c -> p (k g) c")
                NG = 18
                step = NIDX // NG
                for j in range(NG):
                    nc.gpsimd.dma_gather(
                        out_ap=gflat[:, j * (step // 128):(j + 1) * (step // 128), :],
                        in_ap=utab_d[h][:],
                        idxs_ap=idx16[:, j * (step // 16):(j + 1) * (step // 16)].bitcast(I16),
                        num_idxs=step,
                        num_idxs_reg=step,
                        elem_size=DK,
                    )
                gb = [gball[:, kk, :, :] for kk in range(K)]
                nc.gpsimd.tensor_add(gb[0], gb[0], gb[1])
                nc.gpsimd.tensor_add(gb[2], gb[2], gb[3])
                nc.gpsimd.tensor_add(gb[4], gb[4], gb[5])
                nc.gpsimd.tensor_add(gb[6], gb[6], gb[7])
                nc.gpsimd.tensor_add(gb[0], gb[0], gb[4])
                nc.gpsimd.tensor_add(gb[2], gb[2], gb[6])
                nc.gpsimd.tensor_add(gb[0], gb[0], gb[8])
                nc.gpsimd.tensor_add(acc[p][:, :, q * DK:(q + 1) * DK],
                                     gb[0], gb[2])

        # ---- transposes, scramble, Wo ----
        with tc.tile_pool(name="psB", bufs=2, space=bass.MemorySpace.PSUM) as psB:
            outT = [late.tile([128, S], FP32, tag=f"outT{p}", name=f"outT{p}")
                    for p in range(2)]
            for p in range(2):
                for cch in range(NCHUNK):
                    tp = psB.tile([128, 128], FP32, tag="otp")
                    nc.tensor.transpose(tp[:], acc[p][:, cch, :], ident[:])
                    if p == 0:
                        nc.scalar.add(outT[p][:, cch * 128:(cch + 1) * 128], tp[:],
                                      cb_sb[:, 0:1])
                    else:
                        nc.vector.tensor_scalar_add(
                            outT[p][:, cch * 128:(cch + 1) * 128], tp[:], cb_sb[:, 0:1])
            # scramble: AT_h[c2, 64a+r] = outT_pair[r(+64q), 32*c2+a]
            AT = [late.tile([DK, S], BF16, tag=f"AT{h}", name=f"AT{h}")
                  for h in range(HPD)]
            for h in range(HPD):
                p, q = h // 2, h % 2
                for a in range(32):
                    tp = psB.tile([DK, DK], FP32, tag="atp")
                    nc.tensor.transpose(tp[:], outT[p][q * DK:(q + 1) * DK, a:S:32],
                                        id2[q * DK:(q + 1) * DK, :])
                    if h % 2 == 0:
                        nc.scalar.copy(AT[h][:, a * DK:(a + 1) * DK], tp[:])
                    else:
                        nc.vector.tensor_copy(AT[h][:, a * DK:(a + 1) * DK], tp[:])
            # Wo partial projection (accumulate over 4 heads, K=64 each)
            for cch in range(NCHUNK):
                yp = psB.tile([128, D], FP32, tag="yp")
                for h in range(HPD):
                    nc.tensor.matmul(yp[:], AT[h][:, cch * 128:(cch + 1) * 128],
                                     wo_sb[:, h, :], start=(h == 0), stop=(h == HPD - 1))
                yb = late.tile([128, D], FP32, tag="ybounce", bufs=2, name="yb")
                nc.scalar.copy(yb[:], yp[:])
                nc.sync.dma_start(y_d[cch * 128:(cch + 1) * 128, :], yb[:])
            if debug:
                for h in range(HPD):
                    nc.sync.dma_start(idx_dbg[h][:], IDXR[h][:])
                for p in range(2):
                    nc.sync.dma_start(acc_dbg[p][:], acc[p][:])
                    nc.sync.dma_start(qnT_dbg[p][:], qnT[p][:])
                    nc.sync.dma_start(kT_dbg[p][:], kT[p][:])

    nc.compile()
    if split_waits:
        _split_multi_waits(nc)
    return nc


_CACHED = {}


def _get_program():
    if "nc" not in _CACHED:
        _CACHED["nc"] = build_program()
    return _CACHED["nc"]


def make_in_maps(x, Wq, bq, Wk, bk, Wv, bv, Wo, cw, cb):
    wconv_t = np.concatenate([cw[:, :, kk].T for kk in range(K)], axis=1)  # [64, 576]
    wconv_t2 = np.concatenate([wconv_t, wconv_t], axis=0)                  # [128, 576]
    cb2 = np.concatenate([cb, cb])[:, None]                                # [128, 1]
    in_maps = []
    for d in range(8):
        b, g = d // 2, d % 2
        rows = slice(g * HPD * DK, (g + 1) * HPD * DK)
        bk_l, bv_l = bk[rows], bv[rows]
        bk_pair = np.stack([bk_l[0:128], bk_l[128:256]], axis=1)
        bv_pair = np.stack([bv_l[0:128], bv_l[128:256]], axis=1)
        in_maps.append({
            "xT": np.ascontiguousarray(x[b].T),
            "wq_t": np.ascontiguousarray(Wq[rows].T),
            "wk_t": np.ascontiguousarray(Wk[rows].T),
            "wv_t": np.ascontiguousarray(Wv[rows].T),
            "bq": np.ascontiguousarray(bq[rows][None, :]),
            "bk_pair": np.ascontiguousarray(bk_pair),
            "bv_pair": np.ascontiguousarray(bv_pair),
            "wconv_t2": np.ascontiguousarray(wconv_t2).astype(ml_dtypes.bfloat16),
            "conv_b2": np.ascontiguousarray(cb2),
            "wo_t": np.ascontiguousarray(Wo[:, rows].T).astype(ml_dtypes.bfloat16),
        })
    return in_maps


def kernel(**inputs):
    from concourse.bass_utils import run_bass_kernel_spmd

    x = np.asarray(inputs["x"], np.float32)
    Wq = np.asarray(inputs["Wq_w"], np.float32)
    bq = np.asarray(inputs["Wq_b"], np.float32)
    Wk = np.asarray(inputs["Wk_w"], np.float32)
    bk = np.asarray(inputs["Wk_b"], np.float32)
    Wv = np.asarray(inputs["Wv_w"], np.float32)
    bv = np.asarray(inputs["Wv_b"], np.float32)
    Wo = np.asarray(inputs["Wo_w"], np.float32)
    bo = np.asarray(inputs["Wo_b"], np.float32)
    cw = np.asarray(inputs["conv_w"], np.float32)
    cb = np.asarray(inputs["conv_b"], np.float32)

    nc = _get_program()
    in_maps = make_in_maps(x, Wq, bq, Wk, bk, Wv, bv, Wo, cw, cb)
    res = run_bass_kernel_spmd(nc, in_maps, core_ids=list(range(8)))
    y = np.zeros((B, S, D), np.float32)
    for b in range(B):
        y[b] = res.results[2 * b]["y"] + res.results[2 * b + 1]["y"] + bo[None, :]
    return y


if __name__ == "__main__":
    nc = build_program()
    print("program built ok")
